# revision 1
# baseline (speedup 1.0000x reference)
# Trainium2 Bass kernel for nn_DeformableInception (deformable conv x2 -> concat -> 1x1 conv).
#
# Sharding: data-parallel over batch B=8, one sample per NeuronCore (8 cores).
# Weights replicated. No collectives.
#
# Per-core device pipeline (per sample):
#   - bilinear sampling done as pair-gathers: for each tap/position, the 2x2 corner
#     patch is fetched as two overlapping 512B row-pairs from xT [HW, C] (bf16) in DRAM
#     via SWDGE dma_gather (positions land on partitions).
#   - the 4 corner weights (validity/border-folded, precomputed from the offset maps)
#     are applied as per-partition scalars with tensor_scalar/scalar_tensor_tensor on
#     DVE/GPSIMD, accumulating the 2x2 patch into samp[pos, c].
#   - PE transposes samp -> sampT[c, pos], then the deform conv is PSUM-accumulated
#     matmuls over the 9 taps; the two branch outputs feed the 1x1 fuse conv (also PE).
import sys

sys.path.insert(0, "/opt/trn_rl_repo")

import numpy as np
import ml_dtypes

import concourse.bass as bass
import concourse.mybir as mybir
from concourse.tile import TileContext
from concourse.masks import make_identity
from concourse import bacc
from concourse.bass_utils import run_bass_kernel_spmd

bf16 = ml_dtypes.bfloat16

# problem constants (hardcoded per spec)
B = 8
C = 128
H = W = 64
HW = H * W                 # 4096
COUT = 84
K = 3
PAD = 1
KK = K * K                 # 9
NBR = 2                    # two deformable branches
NTAPS = NBR * KK           # 18
NH = 2                     # process positions in two halves of 2048
HALF = HW // NH            # 2048
NBLK = HALF // 128         # 16 blocks of 128 positions per half
NLISTS = NTAPS * NH        # 36 gather lists, 4096 indices each

P = 128
f32 = mybir.dt.float32
bft = mybir.dt.bfloat16
i16 = mybir.dt.int16

# engine split for the blend: every GP_EVERYth position-block's blend ops go to GPSIMD
# Blend engine split: the Pool (GPSIMD) engine rejects TensorScalarPtr on this
# core version ("Instruction engine check failed (Pool)"), so the whole bilinear
# blend runs on DVE. Keep 0.
import os as _os
GP_EVERY = int(_os.environ.get("KERN_GP_EVERY", "0"))  # 0 = all on DVE
ACT_EVERY = int(_os.environ.get("KERN_ACT_EVERY", "2"))  # every 2nd block's first mul on ACT (HW-validated)
ACC_BUFS = int(_os.environ.get("KERN_ACC_BUFS", "8"))
TPP_BUFS = int(_os.environ.get("KERN_TPP_BUFS", "3"))
GPOOL_BUFS = int(_os.environ.get("KERN_GPOOL_BUFS", "3"))
SAMP_BUFS = int(_os.environ.get("KERN_SAMP_BUFS", "3"))

_CACHE = {}


def _host_precompute(x, dm0, dm1, w0, w1, wf, bfv):
    """Numpy precompute: gather indices + folded bilinear weights, weight repacks."""
    ky = np.repeat(np.arange(K) - PAD, K).astype(np.float32)
    kx = np.tile(np.arange(K) - PAD, K).astype(np.float32)
    base_y = np.arange(H, dtype=np.float32).reshape(1, 1, H, 1)
    base_x = np.arange(W, dtype=np.float32).reshape(1, 1, 1, W)

    idx_all = np.zeros((B, NBR, KK, 2, HW), np.int16)     # [:, :, :, t/b, :]
    w_all = np.zeros((B, NBR, KK, 4, HW), np.float32)     # wtA,wtB,wbA,wbB

    for br, dm in ((0, dm0), (1, dm1)):
        off = dm.reshape(B, KK, 2, H, W)
        py = off[:, :, 0] + base_y + ky.reshape(1, KK, 1, 1)
        px = off[:, :, 1] + base_x + kx.reshape(1, KK, 1, 1)
        y0 = np.floor(py); x0 = np.floor(px)
        wy1 = py - y0; wx1 = px - x0
        wy0 = 1.0 - wy1; wx0 = 1.0 - wx1
        y0i = y0.astype(np.int64); x0i = x0.astype(np.int64)
        xb = np.clip(x0i, 0, W - 2)
        for r, (yi, wy) in enumerate(((y0i, wy0), (y0i + 1, wy1))):
            rowvalid = ((yi >= 0) & (yi < H)).astype(np.float32)
            yc = np.clip(yi, 0, H - 1)
            idx_all[:, br, :, r, :] = (yc * W + xb).reshape(B, KK, HW).astype(np.int16)
            wA = np.zeros_like(wy); wB = np.zeros_like(wy)
            for xi, wx in ((x0i, wx0), (x0i + 1, wx1)):
                colvalid = ((xi >= 0) & (xi < W)).astype(np.float32)
                xc = np.clip(xi, 0, W - 1)
                wc = wy * wx * rowvalid * colvalid
                wA += np.where(xc == xb, wc, 0.0)
                wB += np.where(xc == xb + 1, wc, 0.0)
            w_all[:, br, :, 2 * r + 0, :] = wA.reshape(B, KK, HW)
            w_all[:, br, :, 2 * r + 1, :] = wB.reshape(B, KK, HW)

    # xT [B, HW+1, C] bf16 (pad row so the overlapping pair AP stays in bounds)
    xT = np.transpose(x.reshape(B, C, HW), (0, 2, 1))
    xTp = np.concatenate([xT, np.zeros((B, 1, C), np.float32)], axis=1).astype(bf16)

    # IDX [B, NLISTS, 128, 256] int16: list (br, k, Hh) = top-half ++ bot-half, wrapped
    # (j%16, j//16) and replicated across the 8 gpsimd cores' 16-partition groups.
    seqs = np.zeros((B, NBR, KK, NH, 2, HALF), np.int16)
    for hh in range(NH):
        seqs[:, :, :, hh, 0, :] = idx_all[:, :, :, 0, hh * HALF:(hh + 1) * HALF]
        seqs[:, :, :, hh, 1, :] = idx_all[:, :, :, 1, hh * HALF:(hh + 1) * HALF]
    seqs = seqs.reshape(B, NLISTS, HW)                    # list index L = ((br*KK + k)*NH + hh)
    wrapped = seqs.reshape(B, NLISTS, HW // 16, 16)       # j = col*16 + q
    wrapped = np.transpose(wrapped, (0, 1, 3, 2))         # [B, L, 16, 256]
    IDX = np.broadcast_to(wrapped[:, :, None, :, :], (B, NLISTS, 8, 16, HW // 16))
    IDX = np.ascontiguousarray(IDX.reshape(B, NLISTS, P, HW // 16))

    # WS [B, NLISTS, 128, NBLK*4] f32: per position-block b, 4 corner weights;
    # block b of list (br,k,hh) covers positions hh*HALF + b*128 + p
    wsrc = w_all.reshape(B, NBR, KK, 4, NH, NBLK, P)      # [..., c, hh, b, p]
    WS = np.transpose(wsrc, (0, 1, 2, 4, 6, 5, 3))        # [B, br, k, hh, p, b, c]
    WS = np.ascontiguousarray(WS.reshape(B, NLISTS, P, NBLK * 4), np.float32)

    # W0T [NTAPS, C, COUT] bf16: lhsT per (branch, tap)
    W0T = np.zeros((NTAPS, C, COUT), np.float32)
    for br, w in ((0, w0), (1, w1)):
        for k in range(KK):
            W0T[br * KK + k] = w[:, :, k // K, k % K].T
    W0T = W0T.astype(bf16)

    WFT = np.stack([wf[:, :COUT, 0, 0].T, wf[:, COUT:, 0, 0].T]).astype(bf16)  # [2,84,84]
    BF = bfv.reshape(COUT, 1).astype(np.float32)
    return xTp, IDX, WS, W0T, WFT, BF


def _build_nc():
    nc = bacc.Bacc()
    xT_d = nc.declare_dram_parameter("xT", [HW + 1, C], bft, isOutput=False)
    idx_d = nc.declare_dram_parameter("idx", [NLISTS, P, HW // 16], i16, isOutput=False)
    ws_d = nc.declare_dram_parameter("ws", [NLISTS, P, NBLK * 4], f32, isOutput=False)
    w0_d = nc.declare_dram_parameter("w0t", [NTAPS, C, COUT], bft, isOutput=False)
    wf_d = nc.declare_dram_parameter("wft", [2, COUT, COUT], bft, isOutput=False)
    bf_d = nc.declare_dram_parameter("bfv", [COUT, 1], f32, isOutput=False)
    out_d = nc.declare_dram_parameter("out", [COUT, HW], f32, isOutput=True)

    src_ap = bass.AP(tensor=xT_d, offset=0, ap=[[C, HW], [1, 2 * C]])

    with TileContext(nc) as tc:
        with tc.tile_pool(name="const", bufs=1) as const, \
             tc.tile_pool(name="gp", bufs=GPOOL_BUFS) as gp, \
             tc.tile_pool(name="accp", bufs=ACC_BUFS) as accp, \
             tc.tile_pool(name="sampp", bufs=SAMP_BUFS) as sampp, \
             tc.tile_pool(name="op", bufs=2) as op, \
             tc.tile_pool(name="tpp", bufs=TPP_BUFS, space="PSUM") as tpp, \
             tc.tile_pool(name="bigp", bufs=1, space="PSUM") as bigp:
            ident = const.tile([P, P], bft)
            make_identity(nc, ident[:])
            idx_t = const.tile([P, NLISTS, HW // 16], i16)
            ws_t = const.tile([P, NLISTS, NBLK * 4], f32)
            for L in range(NLISTS):
                nc.sync.dma_start(out=idx_t[:, L, :], in_=idx_d[L])
                nc.sync.dma_start(out=ws_t[:, L, :], in_=ws_d[L])
            w0_t = const.tile([C, NTAPS, COUT], bft)
            for t in range(NTAPS):
                nc.sync.dma_start(out=w0_t[:, t, :], in_=w0_d[t])
            wf_t = const.tile([COUT, 2, COUT], bft)
            nc.sync.dma_start(out=wf_t[:, 0, :], in_=wf_d[0])
            nc.sync.dma_start(out=wf_t[:, 1, :], in_=wf_d[1])
            bf_t = const.tile([COUT, 1], f32)
            nc.sync.dma_start(out=bf_t[:], in_=bf_d[:])

            for hh in range(NH):
                o_sb = []
                for br in range(NBR):
                    out_ps = bigp.tile([COUT, HALF], f32, tag="big")
                    for k in range(KK):
                        L = (br * KK + k) * NH + hh
                        g = gp.tile([P, 2 * NBLK, 2 * C], bft, tag="g")
                        nc.gpsimd.dma_gather(
                            out_ap=g[:], in_ap=src_ap, idxs_ap=idx_t[:, L, :],
                            num_idxs=HW, num_idxs_reg=HW,
                            elem_size=2 * C, elem_step=C, transpose=False,
                            single_packet=False,
                        )
                        sampT = sampp.tile([C, HALF], bft, tag="sampT")
                        for qb in range(NBLK // 4):
                            tp = tpp.tile([C, 512], bft, tag="tp")
                            for j in range(4):
                                b = qb * 4 + j
                                eng = (nc.gpsimd if (GP_EVERY and b % GP_EVERY == GP_EVERY - 1)
                                       else nc.vector)
                                acc = accp.tile([P, C], bft, tag="acc")
                                if ACT_EVERY and b % ACT_EVERY == ACT_EVERY - 1:
                                    # offload the chain's first multiply to ACT
                                    nc.scalar.activation(
                                        out=acc[:], in_=g[:, b, 0:C],
                                        func=mybir.ActivationFunctionType.Identity,
                                        scale=ws_t[:, L, b * 4:b * 4 + 1],
                                    )
                                else:
                                    eng.tensor_scalar(
                                        out=acc[:], in0=g[:, b, 0:C],
                                        scalar1=ws_t[:, L, b * 4:b * 4 + 1], scalar2=None,
                                        op0=mybir.AluOpType.mult,
                                    )
                                for sl, (blk, half0) in enumerate(
                                        ((b, C), (b + NBLK, 0), (b + NBLK, C)), start=1):
                                    eng.scalar_tensor_tensor(
                                        out=acc[:], in0=g[:, blk, half0:half0 + C],
                                        scalar=ws_t[:, L, b * 4 + sl:b * 4 + sl + 1],
                                        in1=acc[:],
                                        op0=mybir.AluOpType.mult, op1=mybir.AluOpType.add,
                                    )
                                nc.tensor.matmul(
                                    out=tp[:, j * P:(j + 1) * P], lhsT=acc[:],
                                    rhs=ident[:], is_transpose=True,
                                    start=(j == 0), stop=(j == 3),
                                )
                            nc.scalar.copy(out=sampT[:, qb * 512:(qb + 1) * 512], in_=tp[:])
                        for cc in range(HALF // 512):
                            nc.tensor.matmul(
                                out=out_ps[:, cc * 512:(cc + 1) * 512],
                                lhsT=w0_t[:, br * KK + k, :],
                                rhs=sampT[:, cc * 512:(cc + 1) * 512],
                                start=(k == 0), stop=(k == KK - 1),
                            )
                    ob = op.tile([COUT, HALF], bft, tag="ob")
                    nc.scalar.copy(out=ob[:], in_=out_ps[:])
                    o_sb.append(ob)
                ps2 = bigp.tile([COUT, HALF], f32, tag="big")
                for cc in range(HALF // 512):
                    sl = slice(cc * 512, (cc + 1) * 512)
                    nc.tensor.matmul(out=ps2[:, sl], lhsT=wf_t[:, 0, :],
                                     rhs=o_sb[0][:, sl], start=True, stop=False)
                    nc.tensor.matmul(out=ps2[:, sl], lhsT=wf_t[:, 1, :],
                                     rhs=o_sb[1][:, sl], start=False, stop=True)
                out_sb = op.tile([COUT, HALF], f32, tag="outsb")
                nc.scalar.activation(
                    out=out_sb[:], in_=ps2[:],
                    func=mybir.ActivationFunctionType.Identity, bias=bf_t[:], scale=1.0,
                )
                nc.sync.dma_start(out=out_d[:, hh * HALF:(hh + 1) * HALF], in_=out_sb[:])
    nc.finalize()
    return nc


def kernel(x, dm0, dm1, w0, w1, wf, bf):
    x = np.asarray(x, np.float32)
    dm0 = np.asarray(dm0, np.float32)
    dm1 = np.asarray(dm1, np.float32)
    w0 = np.asarray(w0, np.float32)
    w1 = np.asarray(w1, np.float32)
    wf = np.asarray(wf, np.float32)
    bfv = np.asarray(bf, np.float32)

    xTp, IDX, WS, W0T, WFT, BF = _host_precompute(x, dm0, dm1, w0, w1, wf, bfv)

    if "nc" not in _CACHE:
        _CACHE["nc"] = _build_nc()
    nc = _CACHE["nc"]

    in_maps = [
        {
            "xT": np.ascontiguousarray(xTp[i]),
            "idx": np.ascontiguousarray(IDX[i]),
            "ws": np.ascontiguousarray(WS[i]),
            "w0t": W0T,
            "wft": WFT,
            "bfv": BF,
        }
        for i in range(B)
    ]
    res = run_bass_kernel_spmd(nc, in_maps, core_ids=list(range(B)),
                               **_CACHE.get("run_kwargs", {}))
    _CACHE["last_results"] = res
    out = np.stack([res.results[i]["out"] for i in range(B)])
    return out.reshape(B, COUT, H, W)



# revision 2
# speedup vs baseline: 1.1347x; 1.1347x over previous
# Trainium2 Bass kernel for nn_DeformableInception (deformable conv x2 -> concat -> 1x1 conv).
#
# Sharding: data-parallel over batch B=8, one sample per NeuronCore (8 cores).
# Weights replicated. No collectives.
#
# Per-core device pipeline (per sample):
#   - x is uploaded pair-interleaved: x_pair[q, c, j] = x[c, q+j] (bf16), so each
#     512B gather descriptor delivers the two x-adjacent pixels of a bilinear
#     corner pair in (channel, pixel) order.
#   - for each of the 36 gather lists (branch x tap x image-half), SWDGE
#     dma_gather lands g[pos, grp, c, j] where grp<16 is the top row of position
#     block grp and grp>=16 the bottom row.
#   - the bilinear blend is three big tensor ops instead of per-corner
#     scalar chains: one tensor_tensor multiply against a stride-0-broadcast
#     weight AP (DVE 2x mode), one packed add folding top+bottom rows (DVE 2x),
#     and one strided add folding the x-pair (DVE or GPSIMD, alternating).
#   - PE transposes samp -> sampT[c, pos]; deform conv is PSUM-accumulated
#     matmuls over the 9 taps; the two branch outputs feed the 1x1 fuse conv.
import sys

sys.path.insert(0, "/opt/trn_rl_repo")

import numpy as np
import ml_dtypes

import concourse.bass as bass
import concourse.mybir as mybir
from concourse.tile import TileContext
from concourse.masks import make_identity
from concourse import bacc
from concourse.bass_utils import run_bass_kernel_spmd

bf16 = ml_dtypes.bfloat16

# problem constants (hardcoded per spec)
B = 8
C = 128
H = W = 64
HW = H * W                 # 4096
COUT = 84
K = 3
PAD = 1
KK = K * K                 # 9
NBR = 2                    # two deformable branches
NTAPS = NBR * KK           # 18
NH = 2                     # process positions in two halves of 2048
HALF = HW // NH            # 2048
NBLK = HALF // 128         # 16 blocks of 128 positions per half
NLISTS = NTAPS * NH        # 36 gather lists, 4096 indices each
NGRP = 2 * NBLK            # 32 j-groups per list (16 top rows, 16 bottom rows)

P = 128
f32 = mybir.dt.float32
bft = mybir.dt.bfloat16
i16 = mybir.dt.int16

import os as _os
# fold-j engine split: every POOL_EVERY-th list's x-pair fold runs on GPSIMD
POOL_EVERY = int(_os.environ.get("KERN_POOL_EVERY", "2"))
GPOOL_BUFS = int(_os.environ.get("KERN_GPOOL_BUFS", "3"))
PROD_BUFS = int(_os.environ.get("KERN_PROD_BUFS", "2"))
M_BUFS = int(_os.environ.get("KERN_M_BUFS", "2"))
SAMP_BUFS = int(_os.environ.get("KERN_SAMP_BUFS", "3"))
TPP_BUFS = int(_os.environ.get("KERN_TPP_BUFS", "3"))

_CACHE = {}


def _host_precompute(x, dm0, dm1, w0, w1, wf, bfv):
    """Numpy precompute: gather indices + folded bilinear weights, weight repacks."""
    ky = np.repeat(np.arange(K) - PAD, K).astype(np.float32)
    kx = np.tile(np.arange(K) - PAD, K).astype(np.float32)
    base_y = np.arange(H, dtype=np.float32).reshape(1, 1, H, 1)
    base_x = np.arange(W, dtype=np.float32).reshape(1, 1, 1, W)

    idx_all = np.zeros((B, NBR, KK, 2, HW), np.int16)     # [:, :, :, t/b, :]
    w_all = np.zeros((B, NBR, KK, 4, HW), np.float32)     # wtA,wtB,wbA,wbB

    for br, dm in ((0, dm0), (1, dm1)):
        off = dm.reshape(B, KK, 2, H, W)
        py = off[:, :, 0] + base_y + ky.reshape(1, KK, 1, 1)
        px = off[:, :, 1] + base_x + kx.reshape(1, KK, 1, 1)
        y0 = np.floor(py); x0 = np.floor(px)
        wy1 = py - y0; wx1 = px - x0
        wy0 = 1.0 - wy1; wx0 = 1.0 - wx1
        y0i = y0.astype(np.int64); x0i = x0.astype(np.int64)
        xb = np.clip(x0i, 0, W - 2)
        for r, (yi, wy) in enumerate(((y0i, wy0), (y0i + 1, wy1))):
            rowvalid = ((yi >= 0) & (yi < H)).astype(np.float32)
            yc = np.clip(yi, 0, H - 1)
            idx_all[:, br, :, r, :] = (yc * W + xb).reshape(B, KK, HW).astype(np.int16)
            wA = np.zeros_like(wy); wB = np.zeros_like(wy)
            for xi, wx in ((x0i, wx0), (x0i + 1, wx1)):
                colvalid = ((xi >= 0) & (xi < W)).astype(np.float32)
                xc = np.clip(xi, 0, W - 1)
                wc = wy * wx * rowvalid * colvalid
                wA += np.where(xc == xb, wc, 0.0)
                wB += np.where(xc == xb + 1, wc, 0.0)
            w_all[:, br, :, 2 * r + 0, :] = wA.reshape(B, KK, HW)
            w_all[:, br, :, 2 * r + 1, :] = wB.reshape(B, KK, HW)

    # x_pair [B, HW, C, 2] bf16: x_pair[q, c, j] = x[c, q + j]
    xT = np.transpose(x.reshape(B, C, HW), (0, 2, 1))
    xTp = np.concatenate([xT, np.zeros((B, 1, C), np.float32)], axis=1)
    x_pair = np.stack([xTp[:, :HW], xTp[:, 1:HW + 1]], axis=-1).astype(bf16)

    # IDX [B, NLISTS, 128, 256] int16: list (br, k, Hh) = top-half ++ bot-half, wrapped
    # (j%16, j//16) and replicated across the 8 gpsimd cores' 16-partition groups.
    seqs = np.zeros((B, NBR, KK, NH, 2, HALF), np.int16)
    for hh in range(NH):
        seqs[:, :, :, hh, 0, :] = idx_all[:, :, :, 0, hh * HALF:(hh + 1) * HALF]
        seqs[:, :, :, hh, 1, :] = idx_all[:, :, :, 1, hh * HALF:(hh + 1) * HALF]
    seqs = seqs.reshape(B, NLISTS, HW)                    # list index L = ((br*KK + k)*NH + hh)
    wrapped = seqs.reshape(B, NLISTS, HW // 16, 16)       # j = col*16 + q
    wrapped = np.transpose(wrapped, (0, 1, 3, 2))         # [B, L, 16, 256]
    IDX = np.broadcast_to(wrapped[:, :, None, :, :], (B, NLISTS, 8, 16, HW // 16))
    IDX = np.ascontiguousarray(IDX.reshape(B, NLISTS, P, HW // 16))

    # W4 [B, NLISTS, 128, NGRP, 2] bf16: per j-group g and pixel j the corner weight;
    # group g<16: top row of block g (corners wtA,wtB); g>=16: bottom row (wbA,wbB)
    wsrc = w_all.reshape(B, NBR, KK, 2, 2, NH, NBLK, P)   # [..., r, x, hh, b, p]
    W4 = np.transpose(wsrc, (0, 1, 2, 5, 7, 3, 6, 4))     # [B, br, k, hh, p, r, b, x]
    W4 = np.ascontiguousarray(W4.reshape(B, NLISTS, P, NGRP, 2), np.float32).astype(bf16)

    # W0T [NTAPS, C, COUT] bf16: lhsT per (branch, tap)
    W0T = np.zeros((NTAPS, C, COUT), np.float32)
    for br, w in ((0, w0), (1, w1)):
        for k in range(KK):
            W0T[br * KK + k] = w[:, :, k // K, k % K].T
    W0T = W0T.astype(bf16)

    WFT = np.stack([wf[:, :COUT, 0, 0].T, wf[:, COUT:, 0, 0].T]).astype(bf16)  # [2,84,84]
    BF = bfv.reshape(COUT, 1).astype(np.float32)
    return x_pair, IDX, W4, W0T, WFT, BF


def _build_nc():
    nc = bacc.Bacc()
    xp_d = nc.declare_dram_parameter("xp", [HW, C, 2], bft, isOutput=False)
    idx_d = nc.declare_dram_parameter("idx", [NLISTS, P, HW // 16], i16, isOutput=False)
    w4_d = nc.declare_dram_parameter("w4", [NLISTS, P, NGRP, 2], bft, isOutput=False)
    w0_d = nc.declare_dram_parameter("w0t", [NTAPS, C, COUT], bft, isOutput=False)
    wf_d = nc.declare_dram_parameter("wft", [2, COUT, COUT], bft, isOutput=False)
    bf_d = nc.declare_dram_parameter("bfv", [COUT, 1], f32, isOutput=False)
    out_d = nc.declare_dram_parameter("out", [COUT, HW], f32, isOutput=True)

    src_ap = bass.AP(tensor=xp_d, offset=0, ap=[[2 * C, HW], [1, 2 * C]])

    with TileContext(nc) as tc:
        with tc.tile_pool(name="const", bufs=1) as const, \
             tc.tile_pool(name="gp", bufs=GPOOL_BUFS) as gp, \
             tc.tile_pool(name="prodp", bufs=PROD_BUFS) as prodp, \
             tc.tile_pool(name="mp", bufs=M_BUFS) as mp, \
             tc.tile_pool(name="sampp", bufs=SAMP_BUFS) as sampp, \
             tc.tile_pool(name="op", bufs=2) as op, \
             tc.tile_pool(name="tpp", bufs=TPP_BUFS, space="PSUM") as tpp, \
             tc.tile_pool(name="bigp", bufs=1, space="PSUM") as bigp:
            ident = const.tile([P, P], bft)
            make_identity(nc, ident[:])
            idx_t = const.tile([P, NLISTS, HW // 16], i16)
            w4_t = const.tile([P, NLISTS, NGRP, 2], bft)
            for L in range(NLISTS):
                nc.sync.dma_start(out=idx_t[:, L, :], in_=idx_d[L])
                nc.sync.dma_start(out=w4_t[:, L, :, :], in_=w4_d[L])
            w0_t = const.tile([C, NTAPS, COUT], bft)
            for t in range(NTAPS):
                nc.sync.dma_start(out=w0_t[:, t, :], in_=w0_d[t])
            wf_t = const.tile([COUT, 2, COUT], bft)
            nc.sync.dma_start(out=wf_t[:, 0, :], in_=wf_d[0])
            nc.sync.dma_start(out=wf_t[:, 1, :], in_=wf_d[1])
            bf_t = const.tile([COUT, 1], f32)
            nc.sync.dma_start(out=bf_t[:], in_=bf_d[:])

            for hh in range(NH):
                o_sb = []
                for br in range(NBR):
                    out_ps = bigp.tile([COUT, HALF], f32, tag="big")
                    for k in range(KK):
                        L = (br * KK + k) * NH + hh
                        g = gp.tile([P, NGRP, C, 2], bft, tag="g")
                        g_gather_view = bass.AP(
                            tensor=g.tensor, offset=g.offset,
                            ap=[g.ap[0], [2 * C, NGRP], [1, 2 * C]])
                        nc.gpsimd.dma_gather(
                            out_ap=g_gather_view, in_ap=src_ap, idxs_ap=idx_t[:, L, :],
                            num_idxs=HW, num_idxs_reg=HW,
                            elem_size=2 * C, elem_step=2 * C, transpose=False,
                            single_packet=False,
                        )
                        # blend: prod = g * w4 (weights broadcast along c via stride-0)
                        prod = prodp.tile([P, NGRP, C, 2], bft, tag="prod")
                        w4v = bass.AP(
                            tensor=w4_t.tensor,
                            offset=w4_t.offset + L * (NGRP * 2),
                            ap=[w4_t.ap[0], [2, NGRP], [0, C], [1, 2]])
                        nc.vector.tensor_tensor(
                            out=prod[:], in0=g[:], in1=w4v, op=mybir.AluOpType.mult)
                        # fold top+bottom rows (packed halves, DVE 2x)
                        m = mp.tile([P, NBLK, C, 2], bft, tag="m")
                        nc.vector.tensor_tensor(
                            out=m[:], in0=prod[:, 0:NBLK, :, :],
                            in1=prod[:, NBLK:NGRP, :, :], op=mybir.AluOpType.add)
                        # fold the x-pair (strided views); alternate DVE / GPSIMD
                        samp = sampp.tile([P, NBLK, C], bft, tag="samp")
                        e0 = bass.AP(tensor=m.tensor, offset=m.offset,
                                     ap=[m.ap[0], [2 * C, NBLK], [2, C]])
                        e1 = bass.AP(tensor=m.tensor, offset=m.offset + 1,
                                     ap=[m.ap[0], [2 * C, NBLK], [2, C]])
                        eng = (nc.gpsimd if (POOL_EVERY and L % POOL_EVERY == POOL_EVERY - 1)
                               else nc.vector)
                        eng.tensor_tensor(out=samp[:], in0=e0, in1=e1,
                                          op=mybir.AluOpType.add)
                        # transpose samp -> sampT [C, pos] via PE
                        sampT = sampp.tile([C, HALF], bft, tag="sampT")
                        for qb in range(NBLK // 4):
                            tp = tpp.tile([C, 512], bft, tag="tp")
                            for j in range(4):
                                b = qb * 4 + j
                                nc.tensor.matmul(
                                    out=tp[:, j * P:(j + 1) * P], lhsT=samp[:, b, :],
                                    rhs=ident[:], is_transpose=True,
                                    start=(j == 0), stop=(j == 3),
                                )
                            nc.scalar.copy(out=sampT[:, qb * 512:(qb + 1) * 512], in_=tp[:])
                        for cc in range(HALF // 512):
                            nc.tensor.matmul(
                                out=out_ps[:, cc * 512:(cc + 1) * 512],
                                lhsT=w0_t[:, br * KK + k, :],
                                rhs=sampT[:, cc * 512:(cc + 1) * 512],
                                start=(k == 0), stop=(k == KK - 1),
                            )
                    ob = op.tile([COUT, HALF], bft, tag="ob")
                    nc.scalar.copy(out=ob[:], in_=out_ps[:])
                    o_sb.append(ob)
                ps2 = bigp.tile([COUT, HALF], f32, tag="big")
                for cc in range(HALF // 512):
                    sl = slice(cc * 512, (cc + 1) * 512)
                    nc.tensor.matmul(out=ps2[:, sl], lhsT=wf_t[:, 0, :],
                                     rhs=o_sb[0][:, sl], start=True, stop=False)
                    nc.tensor.matmul(out=ps2[:, sl], lhsT=wf_t[:, 1, :],
                                     rhs=o_sb[1][:, sl], start=False, stop=True)
                out_sb = op.tile([COUT, HALF], f32, tag="outsb")
                nc.scalar.activation(
                    out=out_sb[:], in_=ps2[:],
                    func=mybir.ActivationFunctionType.Identity, bias=bf_t[:], scale=1.0,
                )
                nc.sync.dma_start(out=out_d[:, hh * HALF:(hh + 1) * HALF], in_=out_sb[:])
    nc.finalize()
    return nc


def kernel(x, dm0, dm1, w0, w1, wf, bf):
    x = np.asarray(x, np.float32)
    dm0 = np.asarray(dm0, np.float32)
    dm1 = np.asarray(dm1, np.float32)
    w0 = np.asarray(w0, np.float32)
    w1 = np.asarray(w1, np.float32)
    wf = np.asarray(wf, np.float32)
    bfv = np.asarray(bf, np.float32)

    x_pair, IDX, W4, W0T, WFT, BF = _host_precompute(x, dm0, dm1, w0, w1, wf, bfv)

    if "nc" not in _CACHE:
        _CACHE["nc"] = _build_nc()
    nc = _CACHE["nc"]

    in_maps = [
        {
            "xp": np.ascontiguousarray(x_pair[i]),
            "idx": np.ascontiguousarray(IDX[i]),
            "w4": np.ascontiguousarray(W4[i]),
            "w0t": W0T,
            "wft": WFT,
            "bfv": BF,
        }
        for i in range(B)
    ]
    res = run_bass_kernel_spmd(nc, in_maps, core_ids=list(range(B)),
                               **_CACHE.get("run_kwargs", {}))
    _CACHE["last_results"] = res
    out = np.stack([res.results[i]["out"] for i in range(B)])
    return out.reshape(B, COUT, H, W)


# revision 9
# speedup vs baseline: 1.5676x; 1.3815x over previous
# Trainium2 Bass kernel for nn_DeformableInception (deformable conv x2 -> concat -> 1x1 conv).
#
# Sharding: data-parallel over batch B=8, one sample per NeuronCore (8 cores).
# Weights replicated. No collectives.
#
# Per-core device pipeline (per sample):
#   - x is uploaded pair-interleaved: x_pair[q, c, j] = x[c, q+j] (bf16), so each
#     512B gather descriptor delivers the two x-adjacent pixels of a bilinear
#     corner pair in (channel, pixel) order.
#   - for each of the 36 gather lists (branch x tap x image-half), SWDGE
#     dma_gather lands g[pos, grp, c, j] where grp<16 is the top row of position
#     block grp and grp>=16 the bottom row.
#   - the bilinear blend is three big tensor ops instead of per-corner
#     scalar chains: one tensor_tensor multiply against a stride-0-broadcast
#     weight AP (DVE 2x mode), one packed add folding top+bottom rows (DVE 2x),
#     and one strided add folding the x-pair (DVE or GPSIMD, alternating).
#   - PE transposes samp -> sampT[c, pos]; deform conv is PSUM-accumulated
#     matmuls over the 9 taps; the two branch outputs feed the 1x1 fuse conv.
import sys

sys.path.insert(0, "/opt/trn_rl_repo")

import numpy as np
import ml_dtypes

import concourse.bass as bass
import concourse.mybir as mybir
from concourse.tile import TileContext
from concourse.masks import make_identity
from concourse import bacc
from concourse.bass_utils import run_bass_kernel_spmd

bf16 = ml_dtypes.bfloat16

# problem constants (hardcoded per spec)
B = 8
C = 128
H = W = 64
HW = H * W                 # 4096
COUT = 84
K = 3
PAD = 1
KK = K * K                 # 9
NBR = 2                    # two deformable branches
NTAPS = NBR * KK           # 18
NH = 2                     # process positions in two halves of 2048
HALF = HW // NH            # 2048
NBLK = HALF // 128         # 16 blocks of 128 positions per half
NLISTS = NTAPS * NH        # 36 gather lists, 4096 indices each
NGRP = 2 * NBLK            # 32 j-groups per list (16 top rows, 16 bottom rows)

P = 128
f32 = mybir.dt.float32
bft = mybir.dt.bfloat16
i16 = mybir.dt.int16

import os as _os
# number of lists whose top+bottom row fold runs on GPSIMD instead of DVE
FOLDR_POOL = int(_os.environ.get("KERN_FOLDR_POOL", "14"))
PREFETCH = int(_os.environ.get("KERN_PREFETCH", "4"))
GPOOL_BUFS = int(_os.environ.get("KERN_GPOOL_BUFS", "0")) or (PREFETCH + 1)
PROD_BUFS = int(_os.environ.get("KERN_PROD_BUFS", "2"))
M_BUFS = int(_os.environ.get("KERN_M_BUFS", "2"))
SAMP_BUFS = int(_os.environ.get("KERN_SAMP_BUFS", "3"))
TPP_BUFS = int(_os.environ.get("KERN_TPP_BUFS", "3"))

_CACHE = {}


def _host_precompute(x, dm0, dm1, w0, w1, wf, bfv):
    """Numpy precompute: gather indices + folded bilinear weights, weight repacks."""
    ky = np.repeat(np.arange(K) - PAD, K).astype(np.float32)
    kx = np.tile(np.arange(K) - PAD, K).astype(np.float32)
    base_y = np.arange(H, dtype=np.float32).reshape(1, 1, H, 1)
    base_x = np.arange(W, dtype=np.float32).reshape(1, 1, 1, W)

    idx_all = np.zeros((B, NBR, KK, 2, HW), np.int16)     # [:, :, :, t/b, :]
    w_all = np.zeros((B, NBR, KK, 4, HW), np.float32)     # wtA,wtB,wbA,wbB

    for br, dm in ((0, dm0), (1, dm1)):
        off = dm.reshape(B, KK, 2, H, W)
        py = off[:, :, 0] + base_y + ky.reshape(1, KK, 1, 1)
        px = off[:, :, 1] + base_x + kx.reshape(1, KK, 1, 1)
        y0 = np.floor(py); x0 = np.floor(px)
        wy1 = py - y0; wx1 = px - x0
        wy0 = 1.0 - wy1; wx0 = 1.0 - wx1
        y0i = y0.astype(np.int64); x0i = x0.astype(np.int64)
        xb = np.clip(x0i, 0, W - 2)
        for r, (yi, wy) in enumerate(((y0i, wy0), (y0i + 1, wy1))):
            rowvalid = ((yi >= 0) & (yi < H)).astype(np.float32)
            yc = np.clip(yi, 0, H - 1)
            idx_all[:, br, :, r, :] = (yc * W + xb).reshape(B, KK, HW).astype(np.int16)
            wA = np.zeros_like(wy); wB = np.zeros_like(wy)
            for xi, wx in ((x0i, wx0), (x0i + 1, wx1)):
                colvalid = ((xi >= 0) & (xi < W)).astype(np.float32)
                xc = np.clip(xi, 0, W - 1)
                wc = wy * wx * rowvalid * colvalid
                wA += np.where(xc == xb, wc, 0.0)
                wB += np.where(xc == xb + 1, wc, 0.0)
            w_all[:, br, :, 2 * r + 0, :] = wA.reshape(B, KK, HW)
            w_all[:, br, :, 2 * r + 1, :] = wB.reshape(B, KK, HW)

    # x_pair [B, HW, C, 2] bf16: x_pair[q, c, j] = x[c, q + j]
    xT = np.transpose(x.reshape(B, C, HW), (0, 2, 1))
    xTp = np.concatenate([xT, np.zeros((B, 1, C), np.float32)], axis=1)
    x_pair = np.stack([xTp[:, :HW], xTp[:, 1:HW + 1]], axis=-1).astype(bf16)

    # IDX [B, NLISTS, 128, 256] int16: list (br, k, Hh) = top-half ++ bot-half, wrapped
    # (j%16, j//16) and replicated across the 8 gpsimd cores' 16-partition groups.
    seqs = np.zeros((B, NBR, KK, NH, 2, HALF), np.int16)
    for hh in range(NH):
        seqs[:, :, :, hh, 0, :] = idx_all[:, :, :, 0, hh * HALF:(hh + 1) * HALF]
        seqs[:, :, :, hh, 1, :] = idx_all[:, :, :, 1, hh * HALF:(hh + 1) * HALF]
    seqs = seqs.reshape(B, NLISTS, HW)                    # list index L = ((br*KK + k)*NH + hh)
    wrapped = seqs.reshape(B, NLISTS, HW // 16, 16)       # j = col*16 + q
    wrapped = np.transpose(wrapped, (0, 1, 3, 2))         # [B, L, 16, 256]
    IDX = np.broadcast_to(wrapped[:, :, None, :, :], (B, NLISTS, 8, 16, HW // 16))
    IDX = np.ascontiguousarray(IDX.reshape(B, NLISTS, P, HW // 16))

    # W4 [B, NLISTS, 128, NGRP, 2] bf16: per j-group g and pixel j the corner weight;
    # group g<16: top row of block g (corners wtA,wtB); g>=16: bottom row (wbA,wbB)
    wsrc = w_all.reshape(B, NBR, KK, 2, 2, NH, NBLK, P)   # [..., r, x, hh, b, p]
    W4 = np.transpose(wsrc, (0, 1, 2, 5, 7, 3, 6, 4))     # [B, br, k, hh, p, r, b, x]
    W4 = np.ascontiguousarray(W4.reshape(B, NLISTS, P, NGRP, 2), np.float32).astype(bf16)

    # W0T [NTAPS, C, COUT] bf16: lhsT per (branch, tap)
    W0T = np.zeros((NTAPS, C, COUT), np.float32)
    for br, w in ((0, w0), (1, w1)):
        for k in range(KK):
            W0T[br * KK + k] = w[:, :, k // K, k % K].T
    W0T = W0T.astype(bf16)

    WFT = np.stack([wf[:, :COUT, 0, 0].T, wf[:, COUT:, 0, 0].T]).astype(bf16)  # [2,84,84]
    BF = bfv.reshape(COUT, 1).astype(np.float32)
    return x_pair, IDX, W4, W0T, WFT, BF


def _build_nc():
    nc = bacc.Bacc()
    xp_d = nc.declare_dram_parameter("xp", [HW, C, 2], bft, isOutput=False)
    idx_d = nc.declare_dram_parameter("idx", [NLISTS, P, HW // 16], i16, isOutput=False)
    w4_d = nc.declare_dram_parameter("w4", [NLISTS, P, NGRP, 2], bft, isOutput=False)
    w0_d = nc.declare_dram_parameter("w0t", [NTAPS, C, COUT], bft, isOutput=False)
    wf_d = nc.declare_dram_parameter("wft", [2, COUT, COUT], bft, isOutput=False)
    bf_d = nc.declare_dram_parameter("bfv", [COUT, 1], f32, isOutput=False)
    out_d = nc.declare_dram_parameter("out", [COUT, HW], f32, isOutput=True)

    src_ap = bass.AP(tensor=xp_d, offset=0, ap=[[2 * C, HW], [1, 2 * C]])

    with TileContext(nc) as tc:
        with tc.tile_pool(name="const", bufs=1) as const, \
             tc.tile_pool(name="gp", bufs=GPOOL_BUFS) as gp, \
             tc.tile_pool(name="prodp", bufs=PROD_BUFS) as prodp, \
             tc.tile_pool(name="mp", bufs=M_BUFS) as mp, \
             tc.tile_pool(name="sampp", bufs=SAMP_BUFS) as sampp, \
             tc.tile_pool(name="op", bufs=2) as op, \
             tc.tile_pool(name="tpp", bufs=TPP_BUFS, space="PSUM") as tpp, \
             tc.tile_pool(name="bigp", bufs=1, space="PSUM") as bigp:
            ident = const.tile([P, P], bft)
            make_identity(nc, ident[:])
            # single bulk DMA per constant (partition-major reshuffle via APs)
            w4_t = const.tile([P, NLISTS, NGRP, 2], bft)
            w4_src = bass.AP(
                tensor=w4_d, offset=0,
                ap=[[NGRP * 2, P], [P * NGRP * 2, NLISTS], [2, NGRP], [1, 2]])
            nc.sync.dma_start(out=w4_t[:], in_=w4_src)
            w0_t = const.tile([C, NTAPS, COUT], bft)
            w0_src = bass.AP(
                tensor=w0_d, offset=0,
                ap=[[COUT, C], [C * COUT, NTAPS], [1, COUT]])
            nc.sync.dma_start(out=w0_t[:], in_=w0_src)
            wf_t = const.tile([COUT, 2, COUT], bft)
            nc.sync.dma_start(out=wf_t[:, 0, :], in_=wf_d[0])
            nc.sync.dma_start(out=wf_t[:, 1, :], in_=wf_d[1])
            bf_t = const.tile([COUT, 1], f32)
            nc.sync.dma_start(out=bf_t[:], in_=bf_d[:])

            # flattened (hh, br, k) schedule with gathers issued PREFETCH ahead
            seq = [(hh, br, k) for hh in range(NH) for br in range(NBR)
                   for k in range(KK)]
            NSEQ = len(seq)
            g_ring = {}
            o_sb = []
            out_ps = None
            for i in range(NSEQ + PREFETCH):
                if i < NSEQ:
                    hh, br, k = seq[i]
                    L = (br * KK + k) * NH + hh
                    idx_t = gp.tile([P, HW // 16], i16, tag="idx")
                    nc.sync.dma_start(out=idx_t[:], in_=idx_d[L])
                    g = gp.tile([P, NGRP, C, 2], bft, tag="g")
                    g_gather_view = bass.AP(
                        tensor=g.tensor, offset=g.offset,
                        ap=[g.ap[0], [2 * C, NGRP], [1, 2 * C]])
                    nc.gpsimd.dma_gather(
                        out_ap=g_gather_view, in_ap=src_ap, idxs_ap=idx_t[:],
                        num_idxs=HW, num_idxs_reg=HW,
                        elem_size=2 * C, elem_step=2 * C, transpose=False,
                        single_packet=False,
                    )
                    g_ring[i] = g
                if i < PREFETCH:
                    continue
                hh, br, k = seq[i - PREFETCH]
                L = (br * KK + k) * NH + hh
                g = g_ring.pop(i - PREFETCH)
                if k == 0:
                    out_ps = bigp.tile([COUT, HALF], f32, tag="big")
                # blend: prod = g * w4 (weights broadcast along c via stride-0)
                prod = prodp.tile([P, NGRP, C, 2], bft, tag="prod")
                w4v = bass.AP(
                    tensor=w4_t.tensor,
                    offset=w4_t.offset + L * (NGRP * 2),
                    ap=[w4_t.ap[0], [2, NGRP], [0, C], [1, 2]])
                nc.vector.tensor_tensor(
                    out=prod[:], in0=g[:], in1=w4v, op=mybir.AluOpType.mult)
                # fold top+bottom rows (packed halves, DVE 2x / GPSIMD split)
                m = mp.tile([P, NBLK, C, 2], bft, tag="m")
                ii = i - PREFETCH
                eng = (nc.gpsimd
                       if (ii * FOLDR_POOL) % NSEQ < FOLDR_POOL else nc.vector)
                eng.tensor_tensor(
                    out=m[:], in0=prod[:, 0:NBLK, :, :],
                    in1=prod[:, NBLK:NGRP, :, :], op=mybir.AluOpType.add)
                # fold the x-pair during the PE transpose: per block, two
                # PSUM-accumulated regular matmuls against identity
                sampT = sampp.tile([C, HALF], bft, tag="sampT")
                for qb in range(NBLK // 4):
                    tp = tpp.tile([C, 512], f32, tag="tp")
                    for j in range(4):
                        b = qb * 4 + j
                        e0 = bass.AP(tensor=m.tensor, offset=m.offset + b * 2 * C,
                                     ap=[m.ap[0], [2, C]])
                        e1 = bass.AP(tensor=m.tensor, offset=m.offset + b * 2 * C + 1,
                                     ap=[m.ap[0], [2, C]])
                        nc.tensor.matmul(
                            out=tp[:, j * P:(j + 1) * P], lhsT=e0,
                            rhs=ident[:], start=True, stop=False,
                        )
                        nc.tensor.matmul(
                            out=tp[:, j * P:(j + 1) * P], lhsT=e1,
                            rhs=ident[:], start=False, stop=True,
                        )
                    nc.scalar.copy(out=sampT[:, qb * 512:(qb + 1) * 512], in_=tp[:])
                for cc in range(HALF // 512):
                    nc.tensor.matmul(
                        out=out_ps[:, cc * 512:(cc + 1) * 512],
                        lhsT=w0_t[:, br * KK + k, :],
                        rhs=sampT[:, cc * 512:(cc + 1) * 512],
                        start=(k == 0), stop=(k == KK - 1),
                    )
                if k != KK - 1:
                    continue
                ob = op.tile([COUT, HALF], bft, tag="ob")
                nc.scalar.copy(out=ob[:], in_=out_ps[:])
                o_sb.append(ob)
                if br != NBR - 1:
                    continue
                ps2 = bigp.tile([COUT, HALF], f32, tag="big")
                for cc in range(HALF // 512):
                    sl = slice(cc * 512, (cc + 1) * 512)
                    nc.tensor.matmul(out=ps2[:, sl], lhsT=wf_t[:, 0, :],
                                     rhs=o_sb[0][:, sl], start=True, stop=False)
                    nc.tensor.matmul(out=ps2[:, sl], lhsT=wf_t[:, 1, :],
                                     rhs=o_sb[1][:, sl], start=False, stop=True)
                o_sb = []
                out_sb = op.tile([COUT, HALF], f32, tag="outsb")
                nc.scalar.activation(
                    out=out_sb[:], in_=ps2[:],
                    func=mybir.ActivationFunctionType.Identity, bias=bf_t[:], scale=1.0,
                )
                nc.sync.dma_start(out=out_d[:, hh * HALF:(hh + 1) * HALF], in_=out_sb[:])
    nc.finalize()
    return nc


def kernel(x, dm0, dm1, w0, w1, wf, bf):
    x = np.asarray(x, np.float32)
    dm0 = np.asarray(dm0, np.float32)
    dm1 = np.asarray(dm1, np.float32)
    w0 = np.asarray(w0, np.float32)
    w1 = np.asarray(w1, np.float32)
    wf = np.asarray(wf, np.float32)
    bfv = np.asarray(bf, np.float32)

    x_pair, IDX, W4, W0T, WFT, BF = _host_precompute(x, dm0, dm1, w0, w1, wf, bfv)

    if "nc" not in _CACHE:
        _CACHE["nc"] = _build_nc()
    nc = _CACHE["nc"]

    in_maps = [
        {
            "xp": np.ascontiguousarray(x_pair[i]),
            "idx": np.ascontiguousarray(IDX[i]),
            "w4": np.ascontiguousarray(W4[i]),
            "w0t": W0T,
            "wft": WFT,
            "bfv": BF,
        }
        for i in range(B)
    ]
    res = run_bass_kernel_spmd(nc, in_maps, core_ids=list(range(B)),
                               **_CACHE.get("run_kwargs", {}))
    _CACHE["last_results"] = res
    out = np.stack([res.results[i]["out"] for i in range(B)])
    return out.reshape(B, COUT, H, W)


# revision 26
# speedup vs baseline: 1.6683x; 1.0642x over previous
# Trainium2 Bass kernel for nn_DeformableInception (deformable conv x2 -> concat -> 1x1 conv).
#
# Sharding: data-parallel over batch B=8, one sample per NeuronCore (8 cores).
# Weights replicated. No collectives.
#
# Per-core device pipeline (per sample):
#   - x is uploaded pair-interleaved: x_pair[q, c, j] = x[c, q+j] (bf16), so each
#     512B gather descriptor delivers the two x-adjacent pixels of a bilinear
#     corner pair in (channel, pixel) order.
#   - for each of the 36 gather lists (branch x tap x image-half), SWDGE
#     dma_gather lands g[pos, grp, c, j] where grp<16 is the top row of position
#     block grp and grp>=16 the bottom row.
#   - the bilinear blend is three big tensor ops instead of per-corner
#     scalar chains: one tensor_tensor multiply against a stride-0-broadcast
#     weight AP (DVE 2x mode), one packed add folding top+bottom rows (DVE 2x),
#     and one strided add folding the x-pair (DVE or GPSIMD, alternating).
#   - PE transposes samp -> sampT[c, pos]; deform conv is PSUM-accumulated
#     matmuls over the 9 taps; the two branch outputs feed the 1x1 fuse conv.
import sys

sys.path.insert(0, "/opt/trn_rl_repo")

import numpy as np
import ml_dtypes

import concourse.bass as bass
import concourse.mybir as mybir
from concourse.tile import TileContext
from concourse.masks import make_identity
from concourse import bacc
from concourse.bass_utils import run_bass_kernel_spmd

bf16 = ml_dtypes.bfloat16

# problem constants (hardcoded per spec)
B = 8
C = 128
H = W = 64
HW = H * W                 # 4096
COUT = 84
K = 3
PAD = 1
KK = K * K                 # 9
NBR = 2                    # two deformable branches
NTAPS = NBR * KK           # 18
NH = 2                     # process positions in two halves of 2048
HALF = HW // NH            # 2048
NBLK = HALF // 128         # 16 blocks of 128 positions per half
NLISTS = NTAPS * NH        # 36 gather lists, 4096 indices each
NGRP = 2 * NBLK            # 32 j-groups per list (16 top rows, 16 bottom rows)

P = 128
f32 = mybir.dt.float32
bft = mybir.dt.bfloat16
i16 = mybir.dt.int16

import os as _os
# number of lists whose top+bottom row fold runs on GPSIMD instead of DVE
FOLDR_POOL = int(_os.environ.get("KERN_FOLDR_POOL", "14"))
PREFETCH = int(_os.environ.get("KERN_PREFETCH", "4"))
GPOOL_BUFS = int(_os.environ.get("KERN_GPOOL_BUFS", "0")) or (PREFETCH + 1)
PROD_BUFS = int(_os.environ.get("KERN_PROD_BUFS", "2"))
M_BUFS = int(_os.environ.get("KERN_M_BUFS", "2"))
SAMP_BUFS = int(_os.environ.get("KERN_SAMP_BUFS", "3"))
TPP_BUFS = int(_os.environ.get("KERN_TPP_BUFS", "3"))

_CACHE = {}


def _host_precompute(x, dm0, dm1, w0, w1, wf, bfv):
    """Numpy precompute: gather indices + folded bilinear weights, weight repacks."""
    ky = np.repeat(np.arange(K) - PAD, K).astype(np.float32)
    kx = np.tile(np.arange(K) - PAD, K).astype(np.float32)
    base_y = np.arange(H, dtype=np.float32).reshape(1, 1, H, 1)
    base_x = np.arange(W, dtype=np.float32).reshape(1, 1, 1, W)

    idx_all = np.zeros((B, NBR, KK, 2, HW), np.int16)     # [:, :, :, t/b, :]
    w_all = np.zeros((B, NBR, KK, 4, HW), np.float32)     # wtA,wtB,wbA,wbB

    for br, dm in ((0, dm0), (1, dm1)):
        off = dm.reshape(B, KK, 2, H, W)
        py = off[:, :, 0] + base_y + ky.reshape(1, KK, 1, 1)
        px = off[:, :, 1] + base_x + kx.reshape(1, KK, 1, 1)
        y0 = np.floor(py); x0 = np.floor(px)
        wy1 = py - y0; wx1 = px - x0
        wy0 = 1.0 - wy1; wx0 = 1.0 - wx1
        y0i = y0.astype(np.int64); x0i = x0.astype(np.int64)
        xb = np.clip(x0i, 0, W - 2)
        for r, (yi, wy) in enumerate(((y0i, wy0), (y0i + 1, wy1))):
            rowvalid = ((yi >= 0) & (yi < H)).astype(np.float32)
            yc = np.clip(yi, 0, H - 1)
            idx_all[:, br, :, r, :] = (yc * W + xb).reshape(B, KK, HW).astype(np.int16)
            wA = np.zeros_like(wy); wB = np.zeros_like(wy)
            for xi, wx in ((x0i, wx0), (x0i + 1, wx1)):
                colvalid = ((xi >= 0) & (xi < W)).astype(np.float32)
                xc = np.clip(xi, 0, W - 1)
                wc = wy * wx * rowvalid * colvalid
                wA += np.where(xc == xb, wc, 0.0)
                wB += np.where(xc == xb + 1, wc, 0.0)
            w_all[:, br, :, 2 * r + 0, :] = wA.reshape(B, KK, HW)
            w_all[:, br, :, 2 * r + 1, :] = wB.reshape(B, KK, HW)

    # x_pair [B, HW, C, 2] bf16: x_pair[q, c, j] = x[c, q + j]
    xT = np.transpose(x.reshape(B, C, HW), (0, 2, 1))
    xTp = np.concatenate([xT, np.zeros((B, 1, C), np.float32)], axis=1)
    x_pair = np.stack([xTp[:, :HW], xTp[:, 1:HW + 1]], axis=-1).astype(bf16)

    # IDX [B, NLISTS, 128, 256] int16: list (br, k, Hh) = top-half ++ bot-half, wrapped
    # (j%16, j//16) and replicated across the 8 gpsimd cores' 16-partition groups.
    seqs = np.zeros((B, NBR, KK, NH, 2, HALF), np.int16)
    for hh in range(NH):
        seqs[:, :, :, hh, 0, :] = idx_all[:, :, :, 0, hh * HALF:(hh + 1) * HALF]
        seqs[:, :, :, hh, 1, :] = idx_all[:, :, :, 1, hh * HALF:(hh + 1) * HALF]
    seqs = seqs.reshape(B, NLISTS, HW)                    # list index L = ((br*KK + k)*NH + hh)
    wrapped = seqs.reshape(B, NLISTS, HW // 16, 16)       # j = col*16 + q
    wrapped = np.transpose(wrapped, (0, 1, 3, 2))         # [B, L, 16, 256]
    IDX = np.broadcast_to(wrapped[:, :, None, :, :], (B, NLISTS, 8, 16, HW // 16))
    IDX = np.ascontiguousarray(IDX.reshape(B, NLISTS, P, HW // 16))

    # W4 [B, NLISTS, 128, NGRP, 2] bf16: per j-group g and pixel j the corner weight;
    # group g<16: top row of block g (corners wtA,wtB); g>=16: bottom row (wbA,wbB)
    wsrc = w_all.reshape(B, NBR, KK, 2, 2, NH, NBLK, P)   # [..., r, x, hh, b, p]
    W4 = np.transpose(wsrc, (0, 1, 2, 5, 7, 3, 6, 4))     # [B, br, k, hh, p, r, b, x]
    W4 = np.ascontiguousarray(W4.reshape(B, NLISTS, P, NGRP, 2), np.float32).astype(bf16)

    # reorder lists into the device's flattened (hh, br, k) issue order
    order = [(br * KK + k) * NH + hh
             for hh in range(NH) for br in range(NBR) for k in range(KK)]
    IDX = np.ascontiguousarray(IDX[:, order])
    W4 = np.ascontiguousarray(W4[:, order])

    # W0T [NTAPS, C, COUT] bf16: lhsT per (branch, tap), with the 1x1 fuse conv
    # folded in on the host: M_t = (wf_half @ W0_t)^T so the deform convs
    # accumulate the final output directly (both branches share one PSUM tile)
    wfL = wf[:, :COUT, 0, 0]                              # [COUT, COUT]
    wfR = wf[:, COUT:, 0, 0]
    W0T = np.zeros((NTAPS, C, COUT), np.float32)
    for br, (w, wfh) in ((0, (w0, wfL)), (1, (w1, wfR))):
        for k in range(KK):
            W0T[br * KK + k] = (wfh @ w[:, :, k // K, k % K]).T
    W0T = W0T.astype(bf16)

    BF = bfv.reshape(COUT, 1).astype(np.float32)
    return x_pair, IDX, W4, W0T, BF


def _build_nc():
    nc = bacc.Bacc()
    xp_d = nc.declare_dram_parameter("xp", [HW, C, 2], bft, isOutput=False)
    idx_d = nc.declare_dram_parameter("idx", [NLISTS, P, HW // 16], i16, isOutput=False)
    w4_d = nc.declare_dram_parameter("w4", [NLISTS, P, NGRP, 2], bft, isOutput=False)
    w0_d = nc.declare_dram_parameter("w0t", [NTAPS, C, COUT], bft, isOutput=False)
    bf_d = nc.declare_dram_parameter("bfv", [COUT, 1], f32, isOutput=False)
    out_d = nc.declare_dram_parameter("out", [COUT, HW], f32, isOutput=True)

    src_ap = bass.AP(tensor=xp_d, offset=0, ap=[[2 * C, HW], [1, 2 * C]])

    with TileContext(nc) as tc:
        with tc.tile_pool(name="const", bufs=1) as const, \
             tc.tile_pool(name="gp", bufs=GPOOL_BUFS) as gp, \
             tc.tile_pool(name="prodp", bufs=PROD_BUFS) as prodp, \
             tc.tile_pool(name="mp", bufs=M_BUFS) as mp, \
             tc.tile_pool(name="sampp", bufs=SAMP_BUFS) as sampp, \
             tc.tile_pool(name="op", bufs=2) as op, \
             tc.tile_pool(name="tpp", bufs=TPP_BUFS, space="PSUM") as tpp, \
             tc.tile_pool(name="bigp", bufs=1, space="PSUM") as bigp:
            ident = const.tile([P, P], bft)
            make_identity(nc, ident[:])
            # w4 head + idx head first (small), so gather 0 and the first
            # blend start at once; the bulk uploads behind gather 1
            NHEAD = 6
            w4_t = const.tile([P, NLISTS, NGRP, 2], bft)
            w4_head = bass.AP(
                tensor=w4_d, offset=0,
                ap=[[NGRP * 2, P], [P * NGRP * 2, NHEAD], [2, NGRP], [1, 2]])
            nc.sync.dma_start(out=w4_t[:, 0:NHEAD, :, :], in_=w4_head)
            idx_t = const.tile([P, NLISTS, HW // 16], i16)
            idx_head = bass.AP(
                tensor=idx_d, offset=0,
                ap=[[HW // 16, P], [P * (HW // 16), NHEAD], [1, HW // 16]])
            nc.sync.dma_start(out=idx_t[:, 0:NHEAD, :], in_=idx_head)
            w0_t = const.tile([C, NTAPS, COUT], bft)
            bf_t = const.tile([COUT, 1], f32)

            def _upload_consts():
                idx_src = bass.AP(
                    tensor=idx_d, offset=NHEAD * P * (HW // 16),
                    ap=[[HW // 16, P], [P * (HW // 16), NLISTS - NHEAD],
                        [1, HW // 16]])
                nc.sync.dma_start(out=idx_t[:, NHEAD:NLISTS, :], in_=idx_src)
                w4_src = bass.AP(
                    tensor=w4_d, offset=NHEAD * P * NGRP * 2,
                    ap=[[NGRP * 2, P], [P * NGRP * 2, NLISTS - NHEAD],
                        [2, NGRP], [1, 2]])
                nc.sync.dma_start(out=w4_t[:, NHEAD:NLISTS, :, :], in_=w4_src)
                w0_src = bass.AP(
                    tensor=w0_d, offset=0,
                    ap=[[COUT, C], [C * COUT, NTAPS], [1, COUT]])
                nc.sync.dma_start(out=w0_t[:], in_=w0_src)
                nc.sync.dma_start(out=bf_t[:], in_=bf_d[:])

            # flattened (hh, br, k) schedule with gathers issued PREFETCH ahead
            seq = [(hh, br, k) for hh in range(NH) for br in range(NBR)
                   for k in range(KK)]
            NSEQ = len(seq)
            g_ring = {}
            out_ps = None
            for i in range(NSEQ + PREFETCH):
                if i < NSEQ:
                    g = gp.tile([P, NGRP, C, 2], bft, tag="g")
                    g_gather_view = bass.AP(
                        tensor=g.tensor, offset=g.offset,
                        ap=[g.ap[0], [2 * C, NGRP], [1, 2 * C]])
                    nc.gpsimd.dma_gather(
                        out_ap=g_gather_view, in_ap=src_ap, idxs_ap=idx_t[:, i, :],
                        num_idxs=HW, num_idxs_reg=HW,
                        elem_size=2 * C, elem_step=2 * C, transpose=False,
                        single_packet=False,
                    )
                    g_ring[i] = g
                    if i == 1:
                        _upload_consts()
                if i < PREFETCH:
                    continue
                hh, br, k = seq[i - PREFETCH]
                g = g_ring.pop(i - PREFETCH)
                if br == 0 and k == 0:
                    out_ps = bigp.tile([COUT, HALF], f32, tag="big")
                # blend: prod = g * w4 (weights broadcast along c via stride-0)
                prod = prodp.tile([P, NGRP, C, 2], bft, tag="prod")
                w4v = bass.AP(
                    tensor=w4_t.tensor,
                    offset=w4_t.offset + (i - PREFETCH) * (NGRP * 2),
                    ap=[w4_t.ap[0], [2, NGRP], [0, C], [1, 2]])
                nc.vector.tensor_tensor(
                    out=prod[:], in0=g[:], in1=w4v, op=mybir.AluOpType.mult)
                # fold top+bottom rows (packed halves, DVE 2x / GPSIMD split)
                m = mp.tile([P, NBLK, C, 2], bft, tag="m")
                ii = i - PREFETCH
                # Pool folds spread evenly, but the pipeline tail (after the
                # last gather) stays on the faster DVE
                elig = NSEQ - PREFETCH - 1
                pool_fold = ii < elig and (ii * FOLDR_POOL) % elig < FOLDR_POOL
                eng = nc.gpsimd if pool_fold else nc.vector
                eng.tensor_tensor(
                    out=m[:], in0=prod[:, 0:NBLK, :, :],
                    in1=prod[:, NBLK:NGRP, :, :], op=mybir.AluOpType.add)
                # fold the x-pair during the PE transpose: per block, two
                # PSUM-accumulated regular matmuls against identity
                sampT = sampp.tile([C, HALF], bft, tag="sampT")
                for qb in range(NBLK // 4):
                    tp = tpp.tile([C, 512], f32, tag="tp")
                    for j in range(4):
                        b = qb * 4 + j
                        e0 = bass.AP(tensor=m.tensor, offset=m.offset + b * 2 * C,
                                     ap=[m.ap[0], [2, C]])
                        e1 = bass.AP(tensor=m.tensor, offset=m.offset + b * 2 * C + 1,
                                     ap=[m.ap[0], [2, C]])
                        nc.tensor.matmul(
                            out=tp[:, j * P:(j + 1) * P], lhsT=e0,
                            rhs=ident[:], start=True, stop=False,
                        )
                        nc.tensor.matmul(
                            out=tp[:, j * P:(j + 1) * P], lhsT=e1,
                            rhs=ident[:], start=False, stop=True,
                        )
                    nc.scalar.copy(out=sampT[:, qb * 512:(qb + 1) * 512], in_=tp[:])
                first = (br == 0 and k == 0)
                last = (br == NBR - 1 and k == KK - 1)
                for cc in range(HALF // 512):
                    nc.tensor.matmul(
                        out=out_ps[:, cc * 512:(cc + 1) * 512],
                        lhsT=w0_t[:, br * KK + k, :],
                        rhs=sampT[:, cc * 512:(cc + 1) * 512],
                        start=first, stop=last,
                    )
                if not last:
                    continue
                out_sb = op.tile([COUT, HALF], f32, tag="outsb")
                nc.scalar.activation(
                    out=out_sb[:], in_=out_ps[:],
                    func=mybir.ActivationFunctionType.Identity, bias=bf_t[:], scale=1.0,
                )
                nc.sync.dma_start(out=out_d[:, hh * HALF:(hh + 1) * HALF], in_=out_sb[:])
    nc.finalize()
    return nc


def kernel(x, dm0, dm1, w0, w1, wf, bf):
    x = np.asarray(x, np.float32)
    dm0 = np.asarray(dm0, np.float32)
    dm1 = np.asarray(dm1, np.float32)
    w0 = np.asarray(w0, np.float32)
    w1 = np.asarray(w1, np.float32)
    wf = np.asarray(wf, np.float32)
    bfv = np.asarray(bf, np.float32)

    x_pair, IDX, W4, W0T, BF = _host_precompute(x, dm0, dm1, w0, w1, wf, bfv)

    if "nc" not in _CACHE:
        _CACHE["nc"] = _build_nc()
    nc = _CACHE["nc"]

    in_maps = [
        {
            "xp": np.ascontiguousarray(x_pair[i]),
            "idx": np.ascontiguousarray(IDX[i]),
            "w4": np.ascontiguousarray(W4[i]),
            "w0t": W0T,
            "bfv": BF,
        }
        for i in range(B)
    ]
    res = run_bass_kernel_spmd(nc, in_maps, core_ids=list(range(B)),
                               **_CACHE.get("run_kwargs", {}))
    _CACHE["last_results"] = res
    out = np.stack([res.results[i]["out"] for i in range(B)])
    return out.reshape(B, COUT, H, W)


# revision 40
# speedup vs baseline: 1.7002x; 1.0191x over previous
# Trainium2 Bass kernel for nn_DeformableInception (deformable conv x2 -> concat -> 1x1 conv).
#
# Sharding: data-parallel over batch B=8, one sample per NeuronCore (8 cores).
# Weights replicated. No collectives.
#
# Per-core device pipeline (per sample):
#   - x is uploaded pair-interleaved: x_pair[q, c, j] = x[c, q+j] (bf16), so each
#     512B gather descriptor delivers the two x-adjacent pixels of a bilinear
#     corner pair in (channel, pixel) order.
#   - for each of the 36 gather lists (branch x tap x image-half), SWDGE
#     dma_gather lands g[pos, grp, c, j] where grp<16 is the top row of position
#     block grp and grp>=16 the bottom row.
#   - the bilinear blend is three big tensor ops instead of per-corner
#     scalar chains: one tensor_tensor multiply against a stride-0-broadcast
#     weight AP (DVE 2x mode), one packed add folding top+bottom rows (DVE 2x),
#     and one strided add folding the x-pair (DVE or GPSIMD, alternating).
#   - PE transposes samp -> sampT[c, pos]; deform conv is PSUM-accumulated
#     matmuls over the 9 taps; the two branch outputs feed the 1x1 fuse conv.
import sys

sys.path.insert(0, "/opt/trn_rl_repo")

import numpy as np
import ml_dtypes

import concourse.bass as bass
import concourse.mybir as mybir
from concourse.tile import TileContext
from concourse.masks import make_identity
from concourse import bacc
from concourse.bass_utils import run_bass_kernel_spmd

bf16 = ml_dtypes.bfloat16

# problem constants (hardcoded per spec)
B = 8
C = 128
H = W = 64
HW = H * W                 # 4096
COUT = 84
K = 3
PAD = 1
KK = K * K                 # 9
NBR = 2                    # two deformable branches
NTAPS = NBR * KK           # 18
NH = 2                     # process positions in two halves of 2048
HALF = HW // NH            # 2048
NBLK = HALF // 128         # 16 blocks of 128 positions per half
NLISTS = NTAPS * NH        # 36 gather lists, 4096 indices each
NGRP = 2 * NBLK            # 32 j-groups per list (16 top rows, 16 bottom rows)

P = 128
f32 = mybir.dt.float32
bft = mybir.dt.bfloat16
i16 = mybir.dt.int16

import os as _os
# number of lists whose top+bottom row fold runs on GPSIMD instead of DVE
FOLDR_POOL = int(_os.environ.get("KERN_FOLDR_POOL", "14"))
# 1 = both bilinear folds ride the PE as accumulated matmuls (DVE: mult only)
FOLD_PE = int(_os.environ.get("KERN_FOLD_PE", "0"))
PREFETCH = int(_os.environ.get("KERN_PREFETCH", "4"))
GPOOL_BUFS = int(_os.environ.get("KERN_GPOOL_BUFS", "0")) or (PREFETCH + 1)
PROD_BUFS = int(_os.environ.get("KERN_PROD_BUFS", "2"))
M_BUFS = int(_os.environ.get("KERN_M_BUFS", "2"))
SAMP_BUFS = int(_os.environ.get("KERN_SAMP_BUFS", "3"))
TPP_BUFS = int(_os.environ.get("KERN_TPP_BUFS", "3"))

_CACHE = {}


def _host_precompute(x, dm0, dm1, w0, w1, wf, bfv):
    """Numpy precompute: gather indices + folded bilinear weights, weight repacks."""
    ky = np.repeat(np.arange(K) - PAD, K).astype(np.float32)
    kx = np.tile(np.arange(K) - PAD, K).astype(np.float32)
    base_y = np.arange(H, dtype=np.float32).reshape(1, 1, H, 1)
    base_x = np.arange(W, dtype=np.float32).reshape(1, 1, 1, W)

    idx_all = np.zeros((B, NBR, KK, 2, HW), np.int16)     # [:, :, :, t/b, :]
    w_all = np.zeros((B, NBR, KK, 4, HW), np.float32)     # wtA,wtB,wbA,wbB

    for br, dm in ((0, dm0), (1, dm1)):
        off = dm.reshape(B, KK, 2, H, W)
        py = off[:, :, 0] + base_y + ky.reshape(1, KK, 1, 1)
        px = off[:, :, 1] + base_x + kx.reshape(1, KK, 1, 1)
        y0 = np.floor(py); x0 = np.floor(px)
        wy1 = py - y0; wx1 = px - x0
        wy0 = 1.0 - wy1; wx0 = 1.0 - wx1
        y0i = y0.astype(np.int64); x0i = x0.astype(np.int64)
        xb = np.clip(x0i, 0, W - 2)
        for r, (yi, wy) in enumerate(((y0i, wy0), (y0i + 1, wy1))):
            rowvalid = ((yi >= 0) & (yi < H)).astype(np.float32)
            yc = np.clip(yi, 0, H - 1)
            idx_all[:, br, :, r, :] = (yc * W + xb).reshape(B, KK, HW).astype(np.int16)
            wA = np.zeros_like(wy); wB = np.zeros_like(wy)
            for xi, wx in ((x0i, wx0), (x0i + 1, wx1)):
                colvalid = ((xi >= 0) & (xi < W)).astype(np.float32)
                xc = np.clip(xi, 0, W - 1)
                wc = wy * wx * rowvalid * colvalid
                wA += np.where(xc == xb, wc, 0.0)
                wB += np.where(xc == xb + 1, wc, 0.0)
            w_all[:, br, :, 2 * r + 0, :] = wA.reshape(B, KK, HW)
            w_all[:, br, :, 2 * r + 1, :] = wB.reshape(B, KK, HW)

    # x_pair [B, HW, C, 2] bf16: x_pair[q, c, j] = x[c, q + j]
    xT = np.transpose(x.reshape(B, C, HW), (0, 2, 1))
    xTp = np.concatenate([xT, np.zeros((B, 1, C), np.float32)], axis=1)
    x_pair = np.stack([xTp[:, :HW], xTp[:, 1:HW + 1]], axis=-1).astype(bf16)

    # IDX [B, NLISTS, 128, 256] int16: list (br, k, Hh) = top-half ++ bot-half, wrapped
    # (j%16, j//16) and replicated across the 8 gpsimd cores' 16-partition groups.
    seqs = np.zeros((B, NBR, KK, NH, 2, HALF), np.int16)
    for hh in range(NH):
        seqs[:, :, :, hh, 0, :] = idx_all[:, :, :, 0, hh * HALF:(hh + 1) * HALF]
        seqs[:, :, :, hh, 1, :] = idx_all[:, :, :, 1, hh * HALF:(hh + 1) * HALF]
    seqs = seqs.reshape(B, NLISTS, HW)                    # list index L = ((br*KK + k)*NH + hh)
    wrapped = seqs.reshape(B, NLISTS, HW // 16, 16)       # j = col*16 + q
    wrapped = np.transpose(wrapped, (0, 1, 3, 2))         # [B, L, 16, 256]
    IDX = np.broadcast_to(wrapped[:, :, None, :, :], (B, NLISTS, 8, 16, HW // 16))
    IDX = np.ascontiguousarray(IDX.reshape(B, NLISTS, P, HW // 16))

    # W4 [B, NLISTS, 128, NGRP, 2] bf16: per j-group g and pixel j the corner weight;
    # group g<16: top row of block g (corners wtA,wtB); g>=16: bottom row (wbA,wbB)
    wsrc = w_all.reshape(B, NBR, KK, 2, 2, NH, NBLK, P)   # [..., r, x, hh, b, p]
    W4 = np.transpose(wsrc, (0, 1, 2, 5, 7, 3, 6, 4))     # [B, br, k, hh, p, r, b, x]
    W4 = np.ascontiguousarray(W4.reshape(B, NLISTS, P, NGRP, 2), np.float32).astype(bf16)

    # reorder lists into the device's flattened (hh, br, k) issue order
    order = [(br * KK + k) * NH + hh
             for hh in range(NH) for br in range(NBR) for k in range(KK)]
    IDX = np.ascontiguousarray(IDX[:, order])
    W4 = np.ascontiguousarray(W4[:, order])

    # W0T [NTAPS, C, COUT] bf16: lhsT per (branch, tap), with the 1x1 fuse conv
    # folded in on the host: M_t = (wf_half @ W0_t)^T so the deform convs
    # accumulate the final output directly (both branches share one PSUM tile)
    wfL = wf[:, :COUT, 0, 0]                              # [COUT, COUT]
    wfR = wf[:, COUT:, 0, 0]
    W0T = np.zeros((NTAPS, C, COUT), np.float32)
    for br, (w, wfh) in ((0, (w0, wfL)), (1, (w1, wfR))):
        for k in range(KK):
            W0T[br * KK + k] = (wfh @ w[:, :, k // K, k % K]).T
    W0T = W0T.astype(bf16)

    BF = bfv.reshape(COUT, 1).astype(np.float32)
    return x_pair, IDX, W4, W0T, BF


def _build_nc():
    nc = bacc.Bacc()
    xp_d = nc.declare_dram_parameter("xp", [HW, C, 2], bft, isOutput=False)
    idx_d = nc.declare_dram_parameter("idx", [NLISTS, P, HW // 16], i16, isOutput=False)
    w4_d = nc.declare_dram_parameter("w4", [NLISTS, P, NGRP, 2], bft, isOutput=False)
    w0_d = nc.declare_dram_parameter("w0t", [NTAPS, C, COUT], bft, isOutput=False)
    bf_d = nc.declare_dram_parameter("bfv", [COUT, 1], f32, isOutput=False)
    out_d = nc.declare_dram_parameter("out", [COUT, HW], f32, isOutput=True)

    src_ap = bass.AP(tensor=xp_d, offset=0, ap=[[2 * C, HW], [1, 2 * C]])

    with TileContext(nc) as tc:
        with tc.tile_pool(name="const", bufs=1) as const, \
             tc.tile_pool(name="gp", bufs=GPOOL_BUFS) as gp, \
             tc.tile_pool(name="prodp", bufs=PROD_BUFS) as prodp, \
             tc.tile_pool(name="mp", bufs=M_BUFS) as mp, \
             tc.tile_pool(name="sampp", bufs=SAMP_BUFS) as sampp, \
             tc.tile_pool(name="op", bufs=2) as op, \
             tc.tile_pool(name="tpp", bufs=TPP_BUFS, space="PSUM") as tpp, \
             tc.tile_pool(name="bigp", bufs=1, space="PSUM") as bigp:
            ident = const.tile([P, P], bft)
            make_identity(nc, ident[:])
            # idx + w4 for the first NHEAD lists stream per list through ring
            # tiles (ring reuse keeps those copies from racing ahead of the
            # first gathers); the rest upload as one bulk behind gather 1
            NHEAD = 6
            w0_t = const.tile([C, NTAPS, COUT], bft)
            bf_t = const.tile([COUT, 1], f32)
            idx_t = const.tile([P, NLISTS - NHEAD, HW // 16], i16)
            w4_t = const.tile([P, NLISTS - NHEAD, NGRP, 2], bft)

            def _upload_consts():
                idx_src = bass.AP(
                    tensor=idx_d, offset=NHEAD * P * (HW // 16),
                    ap=[[HW // 16, P], [P * (HW // 16), NLISTS - NHEAD],
                        [1, HW // 16]])
                nc.sync.dma_start(out=idx_t[:], in_=idx_src)
                w4_src = bass.AP(
                    tensor=w4_d, offset=NHEAD * P * NGRP * 2,
                    ap=[[NGRP * 2, P], [P * NGRP * 2, NLISTS - NHEAD],
                        [2, NGRP], [1, 2]])
                nc.sync.dma_start(out=w4_t[:], in_=w4_src)
                w0_src = bass.AP(
                    tensor=w0_d, offset=0,
                    ap=[[COUT, C], [C * COUT, NTAPS], [1, COUT]])
                nc.sync.dma_start(out=w0_t[:], in_=w0_src)
                nc.sync.dma_start(out=bf_t[:], in_=bf_d[:])

            # flattened (hh, br, k) schedule with gathers issued PREFETCH ahead
            seq = [(hh, br, k) for hh in range(NH) for br in range(NBR)
                   for k in range(KK)]
            NSEQ = len(seq)
            g_ring = {}
            out_ps = None
            for i in range(NSEQ + PREFETCH):
                if i < NSEQ:
                    if i < NHEAD:
                        idx_s = gp.tile([P, HW // 16], i16, tag="idx")
                        nc.sync.dma_start(out=idx_s[:], in_=idx_d[i])
                        w4_s = gp.tile([P, NGRP, 2], bft, tag="w4")
                        nc.sync.dma_start(out=w4_s[:], in_=w4_d[i])
                        idx_ap = idx_s[:]
                        w4_ref = (w4_s, 0)
                        if i == 1:
                            _upload_consts()
                    else:
                        idx_ap = idx_t[:, i - NHEAD, :]
                        w4_ref = (w4_t, i - NHEAD)
                    g = gp.tile([P, NGRP, C, 2], bft, tag="g")
                    g_gather_view = bass.AP(
                        tensor=g.tensor, offset=g.offset,
                        ap=[g.ap[0], [2 * C, NGRP], [1, 2 * C]])
                    nc.gpsimd.dma_gather(
                        out_ap=g_gather_view, in_ap=src_ap, idxs_ap=idx_ap,
                        num_idxs=HW, num_idxs_reg=HW,
                        elem_size=2 * C, elem_step=2 * C, transpose=False,
                        single_packet=False,
                    )
                    g_ring[i] = (g, w4_ref)
                if i < PREFETCH:
                    continue
                hh, br, k = seq[i - PREFETCH]
                g, (w4_l, w4_off) = g_ring.pop(i - PREFETCH)
                if br == 0 and k == 0:
                    out_ps = bigp.tile([COUT, HALF], f32, tag="big")
                # blend: prod = g * w4 (weights broadcast along c via stride-0)
                prod = prodp.tile([P, NGRP, C, 2], bft, tag="prod")
                w4v = bass.AP(
                    tensor=w4_l.tensor,
                    offset=w4_l.offset + w4_off * (NGRP * 2),
                    ap=[w4_l.ap[0], [2, NGRP], [0, C], [1, 2]])
                nc.vector.tensor_tensor(
                    out=prod[:], in0=g[:], in1=w4v, op=mybir.AluOpType.mult)
                ii = i - PREFETCH
                if FOLD_PE:
                    # both folds ride the PE: per block, four PSUM-accumulated
                    # matmuls against identity transpose and sum the corners
                    sampT = sampp.tile([C, HALF], bft, tag="sampT")
                    for qb in range(NBLK // 4):
                        tp = tpp.tile([C, 512], f32, tag="tp")
                        for j in range(4):
                            b = qb * 4 + j
                            for si, (blk, pix) in enumerate(
                                    ((b, 0), (b, 1), (b + NBLK, 0), (b + NBLK, 1))):
                                sl = bass.AP(
                                    tensor=prod.tensor,
                                    offset=prod.offset + blk * 2 * C + pix,
                                    ap=[prod.ap[0], [2, C]])
                                nc.tensor.matmul(
                                    out=tp[:, j * P:(j + 1) * P], lhsT=sl,
                                    rhs=ident[:], start=(si == 0), stop=(si == 3),
                                )
                        nc.scalar.copy(out=sampT[:, qb * 512:(qb + 1) * 512], in_=tp[:])
                else:
                    # fold top+bottom rows (packed halves, DVE 2x / GPSIMD split)
                    m = mp.tile([P, NBLK, C, 2], bft, tag="m")
                    # Pool folds spread evenly, but the pipeline tail (after
                    # the last gather) stays on the faster DVE
                    elig = NSEQ - PREFETCH - 1
                    pool_fold = ii < elig and (ii * FOLDR_POOL) % elig < FOLDR_POOL
                    eng = nc.gpsimd if pool_fold else nc.vector
                    eng.tensor_tensor(
                        out=m[:], in0=prod[:, 0:NBLK, :, :],
                        in1=prod[:, NBLK:NGRP, :, :], op=mybir.AluOpType.add)
                    # fold the x-pair during the PE transpose: per block, two
                    # PSUM-accumulated regular matmuls against identity
                    sampT = sampp.tile([C, HALF], bft, tag="sampT")
                    for qb in range(NBLK // 4):
                        tp = tpp.tile([C, 512], f32, tag="tp")
                        for j in range(4):
                            b = qb * 4 + j
                            e0 = bass.AP(tensor=m.tensor, offset=m.offset + b * 2 * C,
                                         ap=[m.ap[0], [2, C]])
                            e1 = bass.AP(tensor=m.tensor, offset=m.offset + b * 2 * C + 1,
                                         ap=[m.ap[0], [2, C]])
                            nc.tensor.matmul(
                                out=tp[:, j * P:(j + 1) * P], lhsT=e0,
                                rhs=ident[:], start=True, stop=False,
                            )
                            nc.tensor.matmul(
                                out=tp[:, j * P:(j + 1) * P], lhsT=e1,
                                rhs=ident[:], start=False, stop=True,
                            )
                        nc.scalar.copy(out=sampT[:, qb * 512:(qb + 1) * 512], in_=tp[:])
                first = (br == 0 and k == 0)
                last = (br == NBR - 1 and k == KK - 1)
                if not last:
                    for cc in range(HALF // 512):
                        nc.tensor.matmul(
                            out=out_ps[:, cc * 512:(cc + 1) * 512],
                            lhsT=w0_t[:, br * KK + k, :],
                            rhs=sampT[:, cc * 512:(cc + 1) * 512],
                            start=first, stop=False,
                        )
                    continue
                # last tap: finish each 512-chunk and stream bias + output out
                out_sb = op.tile([COUT, HALF], f32, tag="outsb")
                for cc in range(HALF // 512):
                    sl = slice(cc * 512, (cc + 1) * 512)
                    nc.tensor.matmul(
                        out=out_ps[:, sl], lhsT=w0_t[:, br * KK + k, :],
                        rhs=sampT[:, sl], start=False, stop=True,
                    )
                    nc.scalar.activation(
                        out=out_sb[:, sl], in_=out_ps[:, sl],
                        func=mybir.ActivationFunctionType.Identity, bias=bf_t[:],
                        scale=1.0,
                    )
                    nc.sync.dma_start(
                        out=out_d[:, hh * HALF + cc * 512:hh * HALF + (cc + 1) * 512],
                        in_=out_sb[:, sl])
    nc.finalize()
    return nc


def kernel(x, dm0, dm1, w0, w1, wf, bf):
    x = np.asarray(x, np.float32)
    dm0 = np.asarray(dm0, np.float32)
    dm1 = np.asarray(dm1, np.float32)
    w0 = np.asarray(w0, np.float32)
    w1 = np.asarray(w1, np.float32)
    wf = np.asarray(wf, np.float32)
    bfv = np.asarray(bf, np.float32)

    x_pair, IDX, W4, W0T, BF = _host_precompute(x, dm0, dm1, w0, w1, wf, bfv)

    if "nc" not in _CACHE:
        _CACHE["nc"] = _build_nc()
    nc = _CACHE["nc"]

    in_maps = [
        {
            "xp": np.ascontiguousarray(x_pair[i]),
            "idx": np.ascontiguousarray(IDX[i]),
            "w4": np.ascontiguousarray(W4[i]),
            "w0t": W0T,
            "bfv": BF,
        }
        for i in range(B)
    ]
    res = run_bass_kernel_spmd(nc, in_maps, core_ids=list(range(B)),
                               **_CACHE.get("run_kwargs", {}))
    _CACHE["last_results"] = res
    out = np.stack([res.results[i]["out"] for i in range(B)])
    return out.reshape(B, COUT, H, W)


# revision 44
# speedup vs baseline: 1.7182x; 1.0106x over previous
# Trainium2 Bass kernel for nn_DeformableInception (deformable conv x2 -> concat -> 1x1 conv).
#
# Sharding: data-parallel over batch B=8, one sample per NeuronCore (8 cores).
# Weights replicated. No collectives.
#
# Per-core device pipeline (per sample):
#   - x is uploaded pair-interleaved: x_pair[q, c, j] = x[c, q+j] (bf16), so each
#     512B gather descriptor delivers the two x-adjacent pixels of a bilinear
#     corner pair in (channel, pixel) order.
#   - for each of the 36 gather lists (branch x tap x image-half), SWDGE
#     dma_gather lands g[pos, grp, c, j] where grp<16 is the top row of position
#     block grp and grp>=16 the bottom row.
#   - the bilinear blend is three big tensor ops instead of per-corner
#     scalar chains: one tensor_tensor multiply against a stride-0-broadcast
#     weight AP (DVE 2x mode), one packed add folding top+bottom rows (DVE 2x),
#     and one strided add folding the x-pair (DVE or GPSIMD, alternating).
#   - PE transposes samp -> sampT[c, pos]; deform conv is PSUM-accumulated
#     matmuls over the 9 taps; the two branch outputs feed the 1x1 fuse conv.
import sys

sys.path.insert(0, "/opt/trn_rl_repo")

import numpy as np
import ml_dtypes

import concourse.bass as bass
import concourse.mybir as mybir
from concourse.tile import TileContext
from concourse.masks import make_identity
from concourse import bacc
from concourse.bass_utils import run_bass_kernel_spmd

bf16 = ml_dtypes.bfloat16

# problem constants (hardcoded per spec)
B = 8
C = 128
H = W = 64
HW = H * W                 # 4096
COUT = 84
K = 3
PAD = 1
KK = K * K                 # 9
NBR = 2                    # two deformable branches
NTAPS = NBR * KK           # 18
NH = 2                     # process positions in two halves of 2048
HALF = HW // NH            # 2048
NBLK = HALF // 128         # 16 blocks of 128 positions per half
NLISTS = NTAPS * NH        # 36 gather lists, 4096 indices each
NGRP = 2 * NBLK            # 32 j-groups per list (16 top rows, 16 bottom rows)

P = 128
f32 = mybir.dt.float32
bft = mybir.dt.bfloat16
i16 = mybir.dt.int16

import os as _os
# number of lists whose top+bottom row fold runs on GPSIMD instead of DVE
FOLDR_POOL = int(_os.environ.get("KERN_FOLDR_POOL", "14"))
# 1 = both bilinear folds ride the PE as accumulated matmuls (DVE: mult only)
FOLD_PE = int(_os.environ.get("KERN_FOLD_PE", "0"))
PREFETCH = int(_os.environ.get("KERN_PREFETCH", "4"))
GPOOL_BUFS = int(_os.environ.get("KERN_GPOOL_BUFS", "0")) or (PREFETCH + 1)
PROD_BUFS = int(_os.environ.get("KERN_PROD_BUFS", "2"))
M_BUFS = int(_os.environ.get("KERN_M_BUFS", "2"))
SAMP_BUFS = int(_os.environ.get("KERN_SAMP_BUFS", "3"))
TPP_BUFS = int(_os.environ.get("KERN_TPP_BUFS", "3"))

_CACHE = {}


def _host_precompute(x, dm0, dm1, w0, w1, wf, bfv):
    """Numpy precompute: gather indices + folded bilinear weights, weight repacks."""
    ky = np.repeat(np.arange(K) - PAD, K).astype(np.float32)
    kx = np.tile(np.arange(K) - PAD, K).astype(np.float32)
    base_y = np.arange(H, dtype=np.float32).reshape(1, 1, H, 1)
    base_x = np.arange(W, dtype=np.float32).reshape(1, 1, 1, W)

    idx_all = np.zeros((B, NBR, KK, 2, HW), np.int16)     # [:, :, :, t/b, :]
    w_all = np.zeros((B, NBR, KK, 4, HW), np.float32)     # wtA,wtB,wbA,wbB

    for br, dm in ((0, dm0), (1, dm1)):
        off = dm.reshape(B, KK, 2, H, W)
        py = off[:, :, 0] + base_y + ky.reshape(1, KK, 1, 1)
        px = off[:, :, 1] + base_x + kx.reshape(1, KK, 1, 1)
        y0 = np.floor(py); x0 = np.floor(px)
        wy1 = py - y0; wx1 = px - x0
        wy0 = 1.0 - wy1; wx0 = 1.0 - wx1
        y0i = y0.astype(np.int64); x0i = x0.astype(np.int64)
        xb = np.clip(x0i, 0, W - 2)
        for r, (yi, wy) in enumerate(((y0i, wy0), (y0i + 1, wy1))):
            rowvalid = ((yi >= 0) & (yi < H)).astype(np.float32)
            yc = np.clip(yi, 0, H - 1)
            idx_all[:, br, :, r, :] = (yc * W + xb).reshape(B, KK, HW).astype(np.int16)
            wA = np.zeros_like(wy); wB = np.zeros_like(wy)
            for xi, wx in ((x0i, wx0), (x0i + 1, wx1)):
                colvalid = ((xi >= 0) & (xi < W)).astype(np.float32)
                xc = np.clip(xi, 0, W - 1)
                wc = wy * wx * rowvalid * colvalid
                wA += np.where(xc == xb, wc, 0.0)
                wB += np.where(xc == xb + 1, wc, 0.0)
            w_all[:, br, :, 2 * r + 0, :] = wA.reshape(B, KK, HW)
            w_all[:, br, :, 2 * r + 1, :] = wB.reshape(B, KK, HW)

    # x_pair [B, HW, C, 2] bf16: x_pair[q, c, j] = x[c, q + j]
    xT = np.transpose(x.reshape(B, C, HW), (0, 2, 1))
    xTp = np.concatenate([xT, np.zeros((B, 1, C), np.float32)], axis=1)
    x_pair = np.stack([xTp[:, :HW], xTp[:, 1:HW + 1]], axis=-1).astype(bf16)

    # IDX [B, NLISTS, 128, 256] int16: list (br, k, Hh) = top-half ++ bot-half, wrapped
    # (j%16, j//16) and replicated across the 8 gpsimd cores' 16-partition groups.
    seqs = np.zeros((B, NBR, KK, NH, 2, HALF), np.int16)
    for hh in range(NH):
        seqs[:, :, :, hh, 0, :] = idx_all[:, :, :, 0, hh * HALF:(hh + 1) * HALF]
        seqs[:, :, :, hh, 1, :] = idx_all[:, :, :, 1, hh * HALF:(hh + 1) * HALF]
    seqs = seqs.reshape(B, NLISTS, HW)                    # list index L = ((br*KK + k)*NH + hh)
    wrapped = seqs.reshape(B, NLISTS, HW // 16, 16)       # j = col*16 + q
    wrapped = np.transpose(wrapped, (0, 1, 3, 2))         # [B, L, 16, 256]
    IDX = np.broadcast_to(wrapped[:, :, None, :, :], (B, NLISTS, 8, 16, HW // 16))
    IDX = np.ascontiguousarray(IDX.reshape(B, NLISTS, P, HW // 16))

    # W4 [B, NLISTS, 128, NGRP, 2] bf16: per j-group g and pixel j the corner weight;
    # group g<16: top row of block g (corners wtA,wtB); g>=16: bottom row (wbA,wbB)
    wsrc = w_all.reshape(B, NBR, KK, 2, 2, NH, NBLK, P)   # [..., r, x, hh, b, p]
    W4 = np.transpose(wsrc, (0, 1, 2, 5, 7, 3, 6, 4))     # [B, br, k, hh, p, r, b, x]
    W4 = np.ascontiguousarray(W4.reshape(B, NLISTS, P, NGRP, 2), np.float32).astype(bf16)

    # reorder lists into the device's flattened (hh, br, k) issue order
    order = [(br * KK + k) * NH + hh
             for hh in range(NH) for br in range(NBR) for k in range(KK)]
    IDX = np.ascontiguousarray(IDX[:, order])
    W4 = np.ascontiguousarray(W4[:, order])

    # W0T [NTAPS, C, COUT] bf16: lhsT per (branch, tap), with the 1x1 fuse conv
    # folded in on the host: M_t = (wf_half @ W0_t)^T so the deform convs
    # accumulate the final output directly (both branches share one PSUM tile)
    wfL = wf[:, :COUT, 0, 0]                              # [COUT, COUT]
    wfR = wf[:, COUT:, 0, 0]
    W0T = np.zeros((NTAPS, C, COUT), np.float32)
    for br, (w, wfh) in ((0, (w0, wfL)), (1, (w1, wfR))):
        for k in range(KK):
            W0T[br * KK + k] = (wfh @ w[:, :, k // K, k % K]).T
    W0T = W0T.astype(bf16)

    BF = bfv.reshape(COUT, 1).astype(np.float32)
    return x_pair, IDX, W4, W0T, BF


def _build_nc():
    nc = bacc.Bacc()
    xp_d = nc.declare_dram_parameter("xp", [HW, C, 2], bft, isOutput=False)
    idx_d = nc.declare_dram_parameter("idx", [NLISTS, P, HW // 16], i16, isOutput=False)
    w4_d = nc.declare_dram_parameter("w4", [NLISTS, P, NGRP, 2], bft, isOutput=False)
    w0_d = nc.declare_dram_parameter("w0t", [NTAPS, C, COUT], bft, isOutput=False)
    bf_d = nc.declare_dram_parameter("bfv", [COUT, 1], f32, isOutput=False)
    out_d = nc.declare_dram_parameter("out", [COUT, HW], bft, isOutput=True)

    src_ap = bass.AP(tensor=xp_d, offset=0, ap=[[2 * C, HW], [1, 2 * C]])

    with TileContext(nc) as tc:
        with tc.tile_pool(name="const", bufs=1) as const, \
             tc.tile_pool(name="gp", bufs=GPOOL_BUFS) as gp, \
             tc.tile_pool(name="prodp", bufs=PROD_BUFS) as prodp, \
             tc.tile_pool(name="mp", bufs=M_BUFS) as mp, \
             tc.tile_pool(name="sampp", bufs=SAMP_BUFS) as sampp, \
             tc.tile_pool(name="op", bufs=2) as op, \
             tc.tile_pool(name="tpp", bufs=TPP_BUFS, space="PSUM") as tpp, \
             tc.tile_pool(name="bigp", bufs=1, space="PSUM") as bigp:
            ident = const.tile([P, P], bft)
            make_identity(nc, ident[:])
            # idx + w4 for the first NHEAD lists stream per list through ring
            # tiles (ring reuse keeps those copies from racing ahead of the
            # first gathers); the rest upload as one bulk behind gather 1
            NHEAD = 6
            w0_t = const.tile([C, NTAPS, COUT], bft)
            bf_t = const.tile([COUT, 1], f32)
            idx_t = const.tile([P, NLISTS - NHEAD, HW // 16], i16)
            w4_t = const.tile([P, NLISTS - NHEAD, NGRP, 2], bft)

            def _upload_consts():
                idx_src = bass.AP(
                    tensor=idx_d, offset=NHEAD * P * (HW // 16),
                    ap=[[HW // 16, P], [P * (HW // 16), NLISTS - NHEAD],
                        [1, HW // 16]])
                nc.sync.dma_start(out=idx_t[:], in_=idx_src)
                w4_src = bass.AP(
                    tensor=w4_d, offset=NHEAD * P * NGRP * 2,
                    ap=[[NGRP * 2, P], [P * NGRP * 2, NLISTS - NHEAD],
                        [2, NGRP], [1, 2]])
                nc.sync.dma_start(out=w4_t[:], in_=w4_src)
                w0_src = bass.AP(
                    tensor=w0_d, offset=0,
                    ap=[[COUT, C], [C * COUT, NTAPS], [1, COUT]])
                nc.sync.dma_start(out=w0_t[:], in_=w0_src)
                nc.sync.dma_start(out=bf_t[:], in_=bf_d[:])

            # flattened (hh, br, k) schedule with gathers issued PREFETCH ahead
            seq = [(hh, br, k) for hh in range(NH) for br in range(NBR)
                   for k in range(KK)]
            NSEQ = len(seq)
            g_ring = {}
            out_ps = None
            for i in range(NSEQ + PREFETCH):
                if i < NSEQ:
                    if i < NHEAD:
                        idx_s = gp.tile([P, HW // 16], i16, tag="idx")
                        nc.sync.dma_start(out=idx_s[:], in_=idx_d[i])
                        w4_s = gp.tile([P, NGRP, 2], bft, tag="w4")
                        nc.sync.dma_start(out=w4_s[:], in_=w4_d[i])
                        idx_ap = idx_s[:]
                        w4_ref = (w4_s, 0)
                        if i == 1:
                            _upload_consts()
                    else:
                        idx_ap = idx_t[:, i - NHEAD, :]
                        w4_ref = (w4_t, i - NHEAD)
                    g = gp.tile([P, NGRP, C, 2], bft, tag="g")
                    if i == NSEQ - 1:
                        # split the final gather 3/4 + 1/4 so the drain tail
                        # pipelines (PE chunks 0-1 run behind the last quarter)
                        for gr0, gr1 in ((0, 24), (24, NGRP)):
                            gv = bass.AP(
                                tensor=g.tensor, offset=g.offset + gr0 * 2 * C,
                                ap=[g.ap[0], [2 * C, gr1 - gr0], [1, 2 * C]])
                            nidx = (gr1 - gr0) * P
                            iv = bass.AP(
                                tensor=idx_ap.tensor,
                                offset=idx_ap.offset + gr0 * (P // 16),
                                ap=[idx_ap.ap[0], [1, nidx // 16]])
                            nc.gpsimd.dma_gather(
                                out_ap=gv, in_ap=src_ap, idxs_ap=iv,
                                num_idxs=nidx, num_idxs_reg=nidx,
                                elem_size=2 * C, elem_step=2 * C, transpose=False,
                                single_packet=False,
                            )
                    else:
                        g_gather_view = bass.AP(
                            tensor=g.tensor, offset=g.offset,
                            ap=[g.ap[0], [2 * C, NGRP], [1, 2 * C]])
                        nc.gpsimd.dma_gather(
                            out_ap=g_gather_view, in_ap=src_ap, idxs_ap=idx_ap,
                            num_idxs=HW, num_idxs_reg=HW,
                            elem_size=2 * C, elem_step=2 * C, transpose=False,
                            single_packet=False,
                        )
                    g_ring[i] = (g, w4_ref)
                if i < PREFETCH:
                    continue
                hh, br, k = seq[i - PREFETCH]
                g, (w4_l, w4_off) = g_ring.pop(i - PREFETCH)
                if br == 0 and k == 0:
                    out_ps = bigp.tile([COUT, HALF], f32, tag="big")
                # blend: prod = g * w4 (weights broadcast along c via stride-0)
                prod = prodp.tile([P, NGRP, C, 2], bft, tag="prod")
                mult_parts = (((0, 24), (24, NGRP)) if i - PREFETCH == NSEQ - 1
                              else ((0, NGRP),))
                for gr0, gr1 in mult_parts:
                    w4v = bass.AP(
                        tensor=w4_l.tensor,
                        offset=w4_l.offset + w4_off * (NGRP * 2) + gr0 * 2,
                        ap=[w4_l.ap[0], [2, gr1 - gr0], [0, C], [1, 2]])
                    nc.vector.tensor_tensor(
                        out=prod[:, gr0:gr1, :, :], in0=g[:, gr0:gr1, :, :],
                        in1=w4v, op=mybir.AluOpType.mult)
                ii = i - PREFETCH
                if FOLD_PE:
                    # both folds ride the PE: per block, four PSUM-accumulated
                    # matmuls against identity transpose and sum the corners
                    sampT = sampp.tile([C, HALF], bft, tag="sampT")
                    for qb in range(NBLK // 4):
                        tp = tpp.tile([C, 512], f32, tag="tp")
                        for j in range(4):
                            b = qb * 4 + j
                            for si, (blk, pix) in enumerate(
                                    ((b, 0), (b, 1), (b + NBLK, 0), (b + NBLK, 1))):
                                sl = bass.AP(
                                    tensor=prod.tensor,
                                    offset=prod.offset + blk * 2 * C + pix,
                                    ap=[prod.ap[0], [2, C]])
                                nc.tensor.matmul(
                                    out=tp[:, j * P:(j + 1) * P], lhsT=sl,
                                    rhs=ident[:], start=(si == 0), stop=(si == 3),
                                )
                        nc.scalar.copy(out=sampT[:, qb * 512:(qb + 1) * 512], in_=tp[:])
                else:
                    # fold top+bottom rows (packed halves, DVE 2x / GPSIMD split)
                    m = mp.tile([P, NBLK, C, 2], bft, tag="m")
                    # Pool folds spread evenly, but the pipeline tail (after
                    # the last gather) stays on the faster DVE
                    elig = NSEQ - PREFETCH - 1
                    pool_fold = ii < elig and (ii * FOLDR_POOL) % elig < FOLDR_POOL
                    eng = nc.gpsimd if pool_fold else nc.vector
                    eng.tensor_tensor(
                        out=m[:], in0=prod[:, 0:NBLK, :, :],
                        in1=prod[:, NBLK:NGRP, :, :], op=mybir.AluOpType.add)
                    # fold the x-pair during the PE transpose: per block, two
                    # PSUM-accumulated regular matmuls against identity
                    sampT = sampp.tile([C, HALF], bft, tag="sampT")
                    for qb in range(NBLK // 4):
                        tp = tpp.tile([C, 512], f32, tag="tp")
                        for j in range(4):
                            b = qb * 4 + j
                            e0 = bass.AP(tensor=m.tensor, offset=m.offset + b * 2 * C,
                                         ap=[m.ap[0], [2, C]])
                            e1 = bass.AP(tensor=m.tensor, offset=m.offset + b * 2 * C + 1,
                                         ap=[m.ap[0], [2, C]])
                            nc.tensor.matmul(
                                out=tp[:, j * P:(j + 1) * P], lhsT=e0,
                                rhs=ident[:], start=True, stop=False,
                            )
                            nc.tensor.matmul(
                                out=tp[:, j * P:(j + 1) * P], lhsT=e1,
                                rhs=ident[:], start=False, stop=True,
                            )
                        nc.scalar.copy(out=sampT[:, qb * 512:(qb + 1) * 512], in_=tp[:])
                first = (br == 0 and k == 0)
                last = (br == NBR - 1 and k == KK - 1)
                if not last:
                    for cc in range(HALF // 512):
                        nc.tensor.matmul(
                            out=out_ps[:, cc * 512:(cc + 1) * 512],
                            lhsT=w0_t[:, br * KK + k, :],
                            rhs=sampT[:, cc * 512:(cc + 1) * 512],
                            start=first, stop=False,
                        )
                    continue
                # last tap: finish each 512-chunk and stream bias + output out
                out_sb = op.tile([COUT, HALF], bft, tag="outsb")
                for cc in range(HALF // 512):
                    sl = slice(cc * 512, (cc + 1) * 512)
                    nc.tensor.matmul(
                        out=out_ps[:, sl], lhsT=w0_t[:, br * KK + k, :],
                        rhs=sampT[:, sl], start=False, stop=True,
                    )
                    nc.scalar.activation(
                        out=out_sb[:, sl], in_=out_ps[:, sl],
                        func=mybir.ActivationFunctionType.Identity, bias=bf_t[:],
                        scale=1.0,
                    )
                    nc.sync.dma_start(
                        out=out_d[:, hh * HALF + cc * 512:hh * HALF + (cc + 1) * 512],
                        in_=out_sb[:, sl])
    nc.finalize()
    return nc


def kernel(x, dm0, dm1, w0, w1, wf, bf):
    x = np.asarray(x, np.float32)
    dm0 = np.asarray(dm0, np.float32)
    dm1 = np.asarray(dm1, np.float32)
    w0 = np.asarray(w0, np.float32)
    w1 = np.asarray(w1, np.float32)
    wf = np.asarray(wf, np.float32)
    bfv = np.asarray(bf, np.float32)

    x_pair, IDX, W4, W0T, BF = _host_precompute(x, dm0, dm1, w0, w1, wf, bfv)

    if "nc" not in _CACHE:
        _CACHE["nc"] = _build_nc()
    nc = _CACHE["nc"]

    in_maps = [
        {
            "xp": np.ascontiguousarray(x_pair[i]),
            "idx": np.ascontiguousarray(IDX[i]),
            "w4": np.ascontiguousarray(W4[i]),
            "w0t": W0T,
            "bfv": BF,
        }
        for i in range(B)
    ]
    res = run_bass_kernel_spmd(nc, in_maps, core_ids=list(range(B)),
                               **_CACHE.get("run_kwargs", {}))
    _CACHE["last_results"] = res
    out = np.stack([res.results[i]["out"].astype(np.float32) for i in range(B)])
    return out.reshape(B, COUT, H, W)
